# revision 2
# baseline (speedup 1.0000x reference)
"""3-layer GCN + linear head on 8 TRN2 NeuronCores (Bass/Tile, SPMD).

Self-contained: hardcodes N=50000, E=600000, D=128, DOUT=32, 8 cores.

Algorithm (matches the PyG-style reference):
    src,dst + self-loops; deg = in-degree; dinv = rsqrt(deg)
    norm_e = dinv[src]*dinv[dst]
    layer(h): agg[d] = sum_e norm_e (hW)[src_e]; relu(agg+b)
    out = h3 @ Wl + bl

Device mapping: nodes sharded into 8 contiguous slabs. Per layer: local
Z~ = dinv ⊙ (H @ W) matmul -> AllGather the 50176x128 table -> bulk
dma_gather of source rows (edges sorted by destination) -> segment-sum via
one-hot selection matmuls (sel built on DVE by iota compare) accumulating
per-128-destination-block PSUM -> relu + dinv scale (bias folded in as a
rank-1 matmul) -> PE transpose feeds next layer's lhsT.
"""
import sys
sys.path.insert(0, '/opt/trn_rl_repo')
import numpy as np

import concourse.bass as bass
import concourse.tile as tile
import concourse.mybir as mybir
from concourse import bacc
from concourse.library_config import mlp as mlp_lib

P = 128
GROUP = 1024      # edges per dma_gather (SWDGE ring carveout = 1024 desc)
GCH = GROUP // P
SB = 4            # gather groups per sel-batch
NQ = 4            # SWDGE queues

N_NODES = 50000
N_CORES = 8
DIN = 128
DOUT = 32
HALF = 32768      # int16 table split


def _preprocess(edge_index, N, C, half):
    LOCAL = N // C
    NB = (LOCAL + P - 1) // P
    PADL = NB * P
    TOT = C * PADL

    src = np.asarray(edge_index[0], dtype=np.int64)
    dst = np.asarray(edge_index[1], dtype=np.int64)
    loops = np.arange(N, dtype=np.int64)
    src = np.concatenate([src, loops])
    dst = np.concatenate([dst, loops])

    deg = np.bincount(dst, minlength=N).astype(np.float64)
    dinv = (1.0 / np.sqrt(deg)).astype(np.float32)
    sdeg = np.sqrt(deg).astype(np.float32)

    gsrc = (src // LOCAL) * PADL + (src % LOCAL)
    assert TOT <= 2 * half

    core = dst // LOCAL
    ldst = dst % LOCAL

    per = [[[None, None] for _ in range(NB)] for _ in range(C)]
    for c in range(C):
        m = core == c
        cs, cl = gsrc[m], ldst[m]
        order = np.argsort(cl, kind="stable")
        cs, cl = cs[order], cl[order]
        blk = cl // P
        dl = cl % P
        s = (cs >= half).astype(np.int64)
        for b in range(NB):
            bm = blk == b
            for st in (0, 1):
                sm = bm & (s == st)
                per[c][b][st] = (cs[sm] - st * half, dl[sm])

    cnt = np.zeros((NB, 2), dtype=np.int64)
    for b in range(NB):
        for st in (0, 1):
            mx = max(len(per[c][b][st][0]) for c in range(C))
            cnt[b, st] = (mx + P - 1) // P

    n_chunks = [int(cnt[:, st].sum()) for st in (0, 1)]
    n_chunks_pad = [((n + GCH - 1) // GCH) * GCH if n else 0 for n in n_chunks]

    idx_w, dloc_w = [], []

    def wrap(idx_flat):
        g = len(idx_flat) // GROUP
        w = idx_flat.reshape(g, GROUP // 16, 16)
        w = np.transpose(w, (0, 2, 1))
        return np.tile(w, (1, 8, 1)).astype(np.int16)

    def dlocw(dl_flat):
        g = len(dl_flat) // GROUP
        d = dl_flat.reshape(g, GCH, P)
        return np.transpose(d, (0, 2, 1)).astype(np.float32)

    for c in range(C):
        sidx = [[], []]
        sdl = [[], []]
        for b in range(NB):
            for st in (0, 1):
                want = cnt[b, st] * P
                ii, dd = per[c][b][st]
                padn = want - len(ii)
                sidx[st].append(np.concatenate([ii, np.zeros(padn, np.int64)]))
                sdl[st].append(np.concatenate([dd, -np.ones(padn, np.int64)]))
        iw, dw = [], []
        for st in (0, 1):
            arr_i = np.concatenate(sidx[st]) if sidx[st] else np.zeros(0, np.int64)
            arr_d = np.concatenate(sdl[st]) if sdl[st] else np.zeros(0, np.int64)
            tail = n_chunks_pad[st] * P - len(arr_i)
            arr_i = np.concatenate([arr_i, np.zeros(tail, np.int64)])
            arr_d = np.concatenate([arr_d, -np.ones(tail, np.int64)])
            iw.append(wrap(arr_i))
            dw.append(dlocw(arr_d))
        idx_w.append(iw)
        dloc_w.append(dw)

    return dict(
        LOCAL=LOCAL, NB=NB, PADL=PADL, TOT=TOT, HALF=half, C=C,
        cnt=cnt, n_chunks_pad=n_chunks_pad,
        idx_w=idx_w, dloc_w=dloc_w, dinv=dinv, sdeg=sdeg,
    )


def _host_tensors(pp, x, weights):
    C, LOCAL, PADL, NB = pp["C"], pp["LOCAL"], pp["PADL"], pp["NB"]
    W1, b1, W2, b2, W3, b3, Wl, bl = weights
    iota = np.tile(np.arange(P, dtype=np.float32), (P, 1))
    ident = np.eye(P, dtype=np.float32)
    ones = np.ones((1, P), np.float32)
    maps = []
    for c in range(C):
        xs = np.zeros((PADL, P), np.float32)
        xs[:LOCAL] = x[c * LOCAL:(c + 1) * LOCAL]
        dvl = np.zeros(PADL, np.float32)
        dvl[:LOCAL] = pp["dinv"][c * LOCAL:(c + 1) * LOCAL]
        dv = np.ascontiguousarray(dvl.reshape(NB, P).T)
        rd = np.zeros((1, PADL), np.float32)
        rd[0, :LOCAL] = pp["sdeg"][c * LOCAL:(c + 1) * LOCAL]
        m = {
            "xt": np.ascontiguousarray(xs.T),
            "w1": np.ascontiguousarray(W1, np.float32),
            "w2": np.ascontiguousarray(W2, np.float32),
            "w3": np.ascontiguousarray(W3, np.float32),
            "wl": np.ascontiguousarray(Wl, np.float32),
            "b1": np.asarray(b1, np.float32).reshape(1, -1),
            "b2": np.asarray(b2, np.float32).reshape(1, -1),
            "b3": np.asarray(b3, np.float32).reshape(1, -1),
            "bl": np.asarray(bl, np.float32).reshape(1, -1),
            "dinv_sb": dv, "recipd": rd,
            "iota": iota, "ident": ident, "ones": ones,
        }
        for st in (0, 1):
            if pp["n_chunks_pad"][st]:
                m[f"idx{st}"] = pp["idx_w"][c][st]
                m[f"dloc{st}"] = pp["dloc_w"][c][st]
        maps.append(m)
    return maps


def _build(pp, DOUT_, n_cores):
    NB, PADL, TOT, half = pp["NB"], pp["PADL"], pp["TOT"], pp["HALF"]
    cnt, n_chunks_pad = pp["cnt"], pp["n_chunks_pad"]
    f32 = mybir.dt.float32

    nc = bacc.Bacc("TRN2", target_bir_lowering=False, debug=False,
                   num_devices=n_cores, num_swdge_queues=NQ)

    xt = nc.dram_tensor("xt", [P, PADL], f32, kind="ExternalInput")
    w = [nc.dram_tensor(f"w{i+1}", [P, P], f32, kind="ExternalInput") for i in range(3)]
    wl = nc.dram_tensor("wl", [P, DOUT_], f32, kind="ExternalInput")
    bias = [nc.dram_tensor(f"b{i+1}", [1, P], f32, kind="ExternalInput") for i in range(3)]
    bl = nc.dram_tensor("bl", [1, DOUT_], f32, kind="ExternalInput")
    dinv_sb_d = nc.dram_tensor("dinv_sb", [P, NB], f32, kind="ExternalInput")
    recipd_d = nc.dram_tensor("recipd", [1, PADL], f32, kind="ExternalInput")
    iota_d = nc.dram_tensor("iota", [P, P], f32, kind="ExternalInput")
    ident_d = nc.dram_tensor("ident", [P, P], f32, kind="ExternalInput")
    ones_d = nc.dram_tensor("ones", [1, P], f32, kind="ExternalInput")
    idx_d, dloc_d = [None, None], [None, None]
    for st in (0, 1):
        g = n_chunks_pad[st] // GCH
        if g:
            idx_d[st] = nc.dram_tensor(f"idx{st}", [g, P, GROUP // 16],
                                       mybir.dt.int16, kind="ExternalInput")
            dloc_d[st] = nc.dram_tensor(f"dloc{st}", [g, P, GCH], f32,
                                        kind="ExternalInput")
    out_d = nc.dram_tensor("out", [PADL, DOUT_], f32, kind="ExternalOutput")

    rg = [list(range(n_cores))]

    from contextlib import ExitStack
    with tile.TileContext(nc) as tc, ExitStack() as ctx:
        dram = ctx.enter_context(tc.tile_pool(name="dram", bufs=1, space="DRAM"))
        cpool = ctx.enter_context(tc.tile_pool(name="consts", bufs=1))
        hpool = ctx.enter_context(tc.tile_pool(name="ht", bufs=1))
        mpool = ctx.enter_context(tc.tile_pool(name="msg", bufs=6))
        spool = ctx.enter_context(tc.tile_pool(name="sel", bufs=2))
        dpool = ctx.enter_context(tc.tile_pool(name="dloc", bufs=2))
        ipool = ctx.enter_context(tc.tile_pool(name="idx", bufs=6))
        zpool = ctx.enter_context(tc.tile_pool(name="zt", bufs=3))
        opool = ctx.enter_context(tc.tile_pool(name="outs", bufs=3))
        pz = ctx.enter_context(tc.tile_pool(name="pz", bufs=2, space="PSUM"))
        pa = ctx.enter_context(tc.tile_pool(name="pa", bufs=2, space="PSUM"))
        pt = ctx.enter_context(tc.tile_pool(name="pt", bufs=2, space="PSUM"))

        nc.gpsimd.load_library(mlp_lib)

        def const(dram_t, shape):
            t = cpool.tile(shape, f32, name=dram_t.name + "_sb")
            nc.sync.dma_start(t[:], dram_t[:])
            return t
        w_sb = [const(w[i], [P, P]) for i in range(3)]
        wl_sb = const(wl, [P, DOUT_])
        b_sb = [const(bias[i], [1, P]) for i in range(3)]
        bl_sb = const(bl, [1, DOUT_])
        dinv_sb = const(dinv_sb_d, [P, NB])
        recipd = const(recipd_d, [1, PADL])
        iota = const(iota_d, [P, P])
        ident = const(ident_d, [P, P])
        ones = const(ones_d, [1, P])

        hT = [hpool.tile([P, PADL], f32, name=f"hT{i}") for i in range(2)]
        nc.sync.dma_start(hT[0][:], xt[:])

        slab = [dram.tile([PADL, P], f32, name=f"slab{i}") for i in range(2)]
        table = [tc.tile([TOT, P], f32, space="DRAM", addr_space="Shared",
                         name=f"table{i}")[0] for i in range(2)]

        gq = [0]

        def agg_phase(layer, hTcur, hTnext, table_t, final=False):
            Wm = wl_sb if final else None
            bm = bl_sb if final else b_sb[layer]
            ndout = DOUT_ if final else P
            msg_tiles = [{}, {}]
            sel_tiles = [{}, {}]

            def ensure_group(st, g):
                if g in msg_tiles[st]:
                    return msg_tiles[st][g]
                it = ipool.tile([P, GROUP // 16], mybir.dt.int16, tag="idx")
                nc.sync.dma_start(it[:], idx_d[st][g])
                mt = mpool.tile([P, GROUP], f32, tag="msg")
                base = table_t[0:half, :] if st == 0 else table_t[half:TOT, :]
                nc.gpsimd.dma_gather(
                    out_ap=mt[:].rearrange("p (c e) -> p c e", e=P),
                    in_ap=base,
                    idxs_ap=it[:],
                    num_idxs=GROUP,
                    num_idxs_reg=GROUP,
                    elem_size=P,
                    queue_num=gq[0] % NQ,
                )
                gq[0] += 1
                msg_tiles[st][g] = mt
                return mt

            def ensure_batch(st, bt):
                if bt in sel_tiles[st]:
                    return sel_tiles[st][bt]
                g0 = bt * SB
                ng = min(SB, n_chunks_pad[st] // GCH - g0)
                nchk = ng * GCH
                dlt = dpool.tile([P, nchk], f32, tag="dloc")
                nc.sync.dma_start(
                    dlt[:].rearrange("p (g m) -> p g m", m=GCH),
                    dloc_d[st][g0:g0 + ng].rearrange("g p m -> p g m"),
                )
                st_t = spool.tile([P, nchk * P], f32, tag="sel")
                nc.vector.tensor_tensor(
                    out=st_t[:].rearrange("p (m j) -> p m j", j=P),
                    in0=dlt[:].to_broadcast([P, nchk, P]),
                    in1=iota[:].unsqueeze(1).to_broadcast([P, nchk, P]),
                    op=mybir.AluOpType.is_equal,
                )
                sel_tiles[st][bt] = (st_t, g0)
                return sel_tiles[st][bt]

            cursor = [0, 0]
            for b in range(NB):
                ap = pa.tile([P, ndout], f32, tag="agg")
                first = True
                if not final:
                    for st in (0, 1):
                        for _ in range(int(cnt[b, st])):
                            ci = cursor[st]
                            cursor[st] += 1
                            g, col = ci // GCH, ci % GCH
                            mt = ensure_group(st, g)
                            sl, g0 = ensure_batch(st, g // SB)
                            scol = (g - g0) * GCH + col
                            nc.tensor.matmul(
                                ap[:],
                                lhsT=sl[:, scol * P:(scol + 1) * P],
                                rhs=mt[:, col * P:(col + 1) * P],
                                start=first, stop=False,
                            )
                            first = False
                lhs_b = ones[:] if final else recipd[0:1, b * P:(b + 1) * P]
                if final:
                    nc.tensor.matmul(ap[:], lhsT=hTcur[:, b * P:(b + 1) * P],
                                     rhs=Wm[:], start=first, stop=False)
                    first = False
                nc.tensor.matmul(ap[:], lhsT=lhs_b, rhs=bm[:],
                                 start=first, stop=True)
                if final:
                    ot = opool.tile([P, DOUT_], f32, tag="o")
                    nc.scalar.activation(ot[:], ap[:],
                                         mybir.ActivationFunctionType.Copy)
                    nc.sync.dma_start(out_d[b * P:(b + 1) * P, :], ot[:])
                else:
                    hn = zpool.tile([P, P], f32, tag="hn")
                    nc.scalar.activation(hn[:], ap[:],
                                         mybir.ActivationFunctionType.Relu,
                                         scale=dinv_sb[:, b:b + 1])
                    tp = pt.tile([P, P], f32, tag="tp")
                    nc.tensor.transpose(out=tp[:], in_=hn[:], identity=ident[:])
                    nc.scalar.activation(hTnext[:, b * P:(b + 1) * P], tp[:],
                                         mybir.ActivationFunctionType.Copy)

        for layer in range(3):
            hTcur = hT[layer % 2]
            hTnext = hT[(layer + 1) % 2]
            slab_t = slab[layer % 2]
            table_t = table[layer % 2]
            for b in range(NB):
                zp = pz.tile([P, P], f32, tag="z")
                nc.tensor.matmul(zp[:], lhsT=hTcur[:, b * P:(b + 1) * P],
                                 rhs=w_sb[layer][:], start=True, stop=True)
                zt = zpool.tile([P, P], f32, tag="zt")
                nc.scalar.activation(zt[:], zp[:],
                                     mybir.ActivationFunctionType.Copy,
                                     scale=dinv_sb[:, b:b + 1])
                nc.sync.dma_start(slab_t[b * P:(b + 1) * P, :], zt[:])
            nc.gpsimd.collective_compute(
                "AllGather", mybir.AluOpType.bypass, replica_groups=rg,
                ins=[slab_t.opt()], outs=[table_t.opt()],
            )
            agg_phase(layer, hTcur, hTnext, table_t)
        agg_phase(3, hT[1], None, None, final=True)

    nc.compile()
    return nc


_CACHE = {}


def _get_compiled(edge_index):
    key = hash(np.asarray(edge_index, np.int64).tobytes())
    if key not in _CACHE:
        pp = _preprocess(edge_index, N_NODES, N_CORES, HALF)
        nc = _build(pp, DOUT, N_CORES)
        _CACHE[key] = (pp, nc)
    return _CACHE[key]


_LAST_RUN = {}


def kernel(x, edge_index, W1, b1, W2, b2, W3, b3, Wl, bl):
    x = np.asarray(x, np.float32)
    pp, nc = _get_compiled(edge_index)
    maps = _host_tensors(pp, x, (W1, b1, W2, b2, W3, b3, Wl, bl))

    from concourse.bass_utils import run_bass_kernel_spmd
    res = run_bass_kernel_spmd(nc, maps, core_ids=list(range(N_CORES)))
    LOCAL = pp["LOCAL"]
    out = np.concatenate(
        [np.asarray(res.results[c]["out"])[:LOCAL] for c in range(N_CORES)])
    _LAST_RUN["nc"] = nc
    _LAST_RUN["maps"] = maps
    return out


def _install_ntff_hook():
    """The agent image's antenv lacks axon_hooks; recreate it from the boot
    helper so run_bass_kernel_spmd(trace=True) can capture NTFF profiles."""
    import types
    if "antenv.axon_hooks" in sys.modules:
        return
    mod = types.ModuleType("antenv.axon_hooks")
    _state = {}
    mod.set_axon_ntff_profile_hook = lambda h: _state.__setitem__("h", h)
    mod.get_axon_ntff_profile_hook = lambda: _state.get("h")
    sys.modules["antenv.axon_hooks"] = mod
    import antenv
    antenv.axon_hooks = mod
    from trn_agent_boot.trn_boot import _ntff_profile_via_ctypes
    mod.set_axon_ntff_profile_hook(
        _ntff_profile_via_ctypes("/opt/axon/libaxon_pjrt.so"))


def profile_exec_ns():
    """Re-run the last kernel invocation with NTFF tracing; return exec ns."""
    if "nc" not in _LAST_RUN:
        return None
    _install_ntff_hook()
    from concourse.bass_utils import run_bass_kernel_spmd
    res = run_bass_kernel_spmd(
        _LAST_RUN["nc"], _LAST_RUN["maps"],
        core_ids=list(range(N_CORES)), trace=True,
    )
    _LAST_RUN["trace_res"] = res
    return res.exec_time_ns


# revision 3
# speedup vs baseline: 1.1825x; 1.1825x over previous
"""3-layer GCN + linear head on 8 TRN2 NeuronCores (Bass/Tile, SPMD).

Self-contained: hardcodes N=50000, E=600000, D=128, DOUT=32, 8 cores.

Algorithm (matches the PyG-style reference):
    src,dst + self-loops; deg = in-degree; dinv = rsqrt(deg)
    norm_e = dinv[src]*dinv[dst]
    layer(h): agg[d] = sum_e norm_e (hW)[src_e]; relu(agg+b)
    out = h3 @ Wl + bl

Device mapping: nodes sharded into 8 contiguous slabs. Per layer: local
Z~ = dinv ⊙ (H @ W) matmul -> AllGather the 50176x128 table -> bulk
dma_gather of source rows (edges sorted by destination) -> segment-sum via
one-hot selection matmuls (sel built on DVE by iota compare) accumulating
per-128-destination-block PSUM -> relu + dinv scale (bias folded in as a
rank-1 matmul) -> PE transpose feeds next layer's lhsT.
"""
import sys
sys.path.insert(0, '/opt/trn_rl_repo')
import numpy as np

import concourse.bass as bass
import concourse.tile as tile
import concourse.mybir as mybir
from concourse import bacc
from concourse.library_config import mlp as mlp_lib

P = 128
GROUP = 1024      # edges per dma_gather (SWDGE ring carveout = 1024 desc)
GCH = GROUP // P
SB = 4            # gather groups per sel-batch
NQ = 4            # SWDGE queues

N_NODES = 50000
N_CORES = 8
DIN = 128
DOUT = 32
HALF = 32768      # int16 table split


def _preprocess(edge_index, N, C, half):
    LOCAL = N // C
    NB = (LOCAL + P - 1) // P
    PADL = NB * P
    TOT = C * PADL

    src = np.asarray(edge_index[0], dtype=np.int64)
    dst = np.asarray(edge_index[1], dtype=np.int64)
    loops = np.arange(N, dtype=np.int64)
    src = np.concatenate([src, loops])
    dst = np.concatenate([dst, loops])

    deg = np.bincount(dst, minlength=N).astype(np.float64)
    dinv = (1.0 / np.sqrt(deg)).astype(np.float32)
    sdeg = np.sqrt(deg).astype(np.float32)

    gsrc = (src // LOCAL) * PADL + (src % LOCAL)
    assert TOT <= 2 * half

    core = dst // LOCAL
    ldst = dst % LOCAL

    per = [[[None, None] for _ in range(NB)] for _ in range(C)]
    for c in range(C):
        m = core == c
        cs, cl = gsrc[m], ldst[m]
        order = np.argsort(cl, kind="stable")
        cs, cl = cs[order], cl[order]
        blk = cl // P
        dl = cl % P
        s = (cs >= half).astype(np.int64)
        for b in range(NB):
            bm = blk == b
            for st in (0, 1):
                sm = bm & (s == st)
                per[c][b][st] = (cs[sm] - st * half, dl[sm])

    cnt = np.zeros((NB, 2), dtype=np.int64)
    for b in range(NB):
        for st in (0, 1):
            mx = max(len(per[c][b][st][0]) for c in range(C))
            cnt[b, st] = (mx + P - 1) // P

    n_chunks = [int(cnt[:, st].sum()) for st in (0, 1)]
    n_chunks_pad = [((n + GCH - 1) // GCH) * GCH if n else 0 for n in n_chunks]

    idx_w, dloc_w = [], []

    def wrap(idx_flat):
        g = len(idx_flat) // GROUP
        w = idx_flat.reshape(g, GROUP // 16, 16)
        w = np.transpose(w, (0, 2, 1))
        return np.tile(w, (1, 8, 1)).astype(np.int16)

    def dlocw(dl_flat):
        g = len(dl_flat) // GROUP
        d = dl_flat.reshape(g, GCH, P)
        return np.transpose(d, (0, 2, 1)).astype(np.float32)

    for c in range(C):
        sidx = [[], []]
        sdl = [[], []]
        for b in range(NB):
            for st in (0, 1):
                want = cnt[b, st] * P
                ii, dd = per[c][b][st]
                padn = want - len(ii)
                sidx[st].append(np.concatenate([ii, np.zeros(padn, np.int64)]))
                sdl[st].append(np.concatenate([dd, -np.ones(padn, np.int64)]))
        iw, dw = [], []
        for st in (0, 1):
            arr_i = np.concatenate(sidx[st]) if sidx[st] else np.zeros(0, np.int64)
            arr_d = np.concatenate(sdl[st]) if sdl[st] else np.zeros(0, np.int64)
            tail = n_chunks_pad[st] * P - len(arr_i)
            arr_i = np.concatenate([arr_i, np.zeros(tail, np.int64)])
            arr_d = np.concatenate([arr_d, -np.ones(tail, np.int64)])
            iw.append(wrap(arr_i))
            dw.append(dlocw(arr_d))
        idx_w.append(iw)
        dloc_w.append(dw)

    return dict(
        LOCAL=LOCAL, NB=NB, PADL=PADL, TOT=TOT, HALF=half, C=C,
        cnt=cnt, n_chunks_pad=n_chunks_pad,
        idx_w=idx_w, dloc_w=dloc_w, dinv=dinv, sdeg=sdeg,
    )


def _host_tensors(pp, x, weights):
    C, LOCAL, PADL, NB = pp["C"], pp["LOCAL"], pp["PADL"], pp["NB"]
    W1, b1, W2, b2, W3, b3, Wl, bl = weights
    iota = np.tile(np.arange(P, dtype=np.float32), (P, 1))
    ident = np.eye(P, dtype=np.float32)
    ones = np.ones((1, P), np.float32)
    maps = []
    for c in range(C):
        xs = np.zeros((PADL, P), np.float32)
        xs[:LOCAL] = x[c * LOCAL:(c + 1) * LOCAL]
        dvl = np.zeros(PADL, np.float32)
        dvl[:LOCAL] = pp["dinv"][c * LOCAL:(c + 1) * LOCAL]
        dv = np.ascontiguousarray(dvl.reshape(NB, P).T)
        rd = np.zeros((1, PADL), np.float32)
        rd[0, :LOCAL] = pp["sdeg"][c * LOCAL:(c + 1) * LOCAL]
        m = {
            "xt": np.ascontiguousarray(xs.T),
            "w1": np.ascontiguousarray(W1, np.float32),
            "w2": np.ascontiguousarray(W2, np.float32),
            "w3": np.ascontiguousarray(W3, np.float32),
            "wl": np.ascontiguousarray(Wl, np.float32),
            "b1": np.asarray(b1, np.float32).reshape(1, -1),
            "b2": np.asarray(b2, np.float32).reshape(1, -1),
            "b3": np.asarray(b3, np.float32).reshape(1, -1),
            "bl": np.asarray(bl, np.float32).reshape(1, -1),
            "dinv_sb": dv, "recipd": rd,
            "iota": iota, "ident": ident, "ones": ones,
        }
        for st in (0, 1):
            if pp["n_chunks_pad"][st]:
                m[f"idx{st}"] = pp["idx_w"][c][st]
                m[f"dloc{st}"] = pp["dloc_w"][c][st]
        maps.append(m)
    return maps


def _build(pp, DOUT_, n_cores):
    NB, PADL, TOT, half = pp["NB"], pp["PADL"], pp["TOT"], pp["HALF"]
    cnt, n_chunks_pad = pp["cnt"], pp["n_chunks_pad"]
    f32 = mybir.dt.float32
    f16 = mybir.dt.float16

    nc = bacc.Bacc("TRN2", target_bir_lowering=False, debug=False,
                   num_devices=n_cores, num_swdge_queues=NQ)

    xt = nc.dram_tensor("xt", [P, PADL], f32, kind="ExternalInput")
    w = [nc.dram_tensor(f"w{i+1}", [P, P], f32, kind="ExternalInput") for i in range(3)]
    wl = nc.dram_tensor("wl", [P, DOUT_], f32, kind="ExternalInput")
    bias = [nc.dram_tensor(f"b{i+1}", [1, P], f32, kind="ExternalInput") for i in range(3)]
    bl = nc.dram_tensor("bl", [1, DOUT_], f32, kind="ExternalInput")
    dinv_sb_d = nc.dram_tensor("dinv_sb", [P, NB], f32, kind="ExternalInput")
    recipd_d = nc.dram_tensor("recipd", [1, PADL], f32, kind="ExternalInput")
    iota_d = nc.dram_tensor("iota", [P, P], f32, kind="ExternalInput")
    ident_d = nc.dram_tensor("ident", [P, P], f32, kind="ExternalInput")
    ones_d = nc.dram_tensor("ones", [1, P], f32, kind="ExternalInput")
    idx_d, dloc_d = [None, None], [None, None]
    for st in (0, 1):
        g = n_chunks_pad[st] // GCH
        if g:
            idx_d[st] = nc.dram_tensor(f"idx{st}", [g, P, GROUP // 16],
                                       mybir.dt.int16, kind="ExternalInput")
            dloc_d[st] = nc.dram_tensor(f"dloc{st}", [g, P, GCH], f32,
                                        kind="ExternalInput")
    out_d = nc.dram_tensor("out", [PADL, DOUT_], f32, kind="ExternalOutput")

    rg = [list(range(n_cores))]

    from contextlib import ExitStack
    with tile.TileContext(nc) as tc, ExitStack() as ctx:
        dram = ctx.enter_context(tc.tile_pool(name="dram", bufs=1, space="DRAM"))
        cpool = ctx.enter_context(tc.tile_pool(name="consts", bufs=1))
        hpool = ctx.enter_context(tc.tile_pool(name="ht", bufs=1))
        mpool = ctx.enter_context(tc.tile_pool(name="msg", bufs=6))
        spool = ctx.enter_context(tc.tile_pool(name="sel", bufs=2))
        dpool = ctx.enter_context(tc.tile_pool(name="dloc", bufs=2))
        ipool = ctx.enter_context(tc.tile_pool(name="idx", bufs=6))
        zpool = ctx.enter_context(tc.tile_pool(name="zt", bufs=3))
        opool = ctx.enter_context(tc.tile_pool(name="outs", bufs=3))
        pz = ctx.enter_context(tc.tile_pool(name="pz", bufs=2, space="PSUM"))
        pa = ctx.enter_context(tc.tile_pool(name="pa", bufs=2, space="PSUM"))
        pt = ctx.enter_context(tc.tile_pool(name="pt", bufs=2, space="PSUM"))

        nc.gpsimd.load_library(mlp_lib)

        def const(dram_t, shape):
            t = cpool.tile(shape, f32, name=dram_t.name + "_sb")
            nc.sync.dma_start(t[:], dram_t[:])
            return t
        w_sb = [const(w[i], [P, P]) for i in range(3)]
        wl_sb = const(wl, [P, DOUT_])
        b_sb = [const(bias[i], [1, P]) for i in range(3)]
        bl_sb = const(bl, [1, DOUT_])
        dinv_sb = const(dinv_sb_d, [P, NB])
        recipd = const(recipd_d, [1, PADL])
        iota = const(iota_d, [P, P])
        ident = const(ident_d, [P, P])
        ones = const(ones_d, [1, P])

        hT = [hpool.tile([P, PADL], f32, name=f"hT{i}") for i in range(2)]
        nc.sync.dma_start(hT[0][:], xt[:])

        slab = [dram.tile([PADL, P], f16, name=f"slab{i}") for i in range(2)]
        table = [tc.tile([TOT, P], f16, space="DRAM", addr_space="Shared",
                         name=f"table{i}")[0] for i in range(2)]

        gq = [0]

        def agg_phase(layer, hTcur, hTnext, table_t, final=False):
            Wm = wl_sb if final else None
            bm = bl_sb if final else b_sb[layer]
            ndout = DOUT_ if final else P
            msg_tiles = [{}, {}]
            sel_tiles = [{}, {}]

            def ensure_group(st, g):
                if g in msg_tiles[st]:
                    return msg_tiles[st][g]
                it = ipool.tile([P, GROUP // 16], mybir.dt.int16, tag="idx")
                nc.sync.dma_start(it[:], idx_d[st][g])
                mt = mpool.tile([P, GROUP], f16, tag="msg")
                base = table_t[0:half, :] if st == 0 else table_t[half:TOT, :]
                nc.gpsimd.dma_gather(
                    out_ap=mt[:].rearrange("p (c e) -> p c e", e=P),
                    in_ap=base,
                    idxs_ap=it[:],
                    num_idxs=GROUP,
                    num_idxs_reg=GROUP,
                    elem_size=P,
                    queue_num=gq[0] % NQ,
                )
                gq[0] += 1
                msg_tiles[st][g] = mt
                return mt

            def ensure_batch(st, bt):
                if bt in sel_tiles[st]:
                    return sel_tiles[st][bt]
                g0 = bt * SB
                ng = min(SB, n_chunks_pad[st] // GCH - g0)
                nchk = ng * GCH
                dlt = dpool.tile([P, nchk], f32, tag="dloc")
                nc.sync.dma_start(
                    dlt[:].rearrange("p (g m) -> p g m", m=GCH),
                    dloc_d[st][g0:g0 + ng].rearrange("g p m -> p g m"),
                )
                st_t = spool.tile([P, nchk * P], f16, tag="sel")
                nc.vector.tensor_tensor(
                    out=st_t[:].rearrange("p (m j) -> p m j", j=P),
                    in0=dlt[:].to_broadcast([P, nchk, P]),
                    in1=iota[:].unsqueeze(1).to_broadcast([P, nchk, P]),
                    op=mybir.AluOpType.is_equal,
                )
                sel_tiles[st][bt] = (st_t, g0)
                return sel_tiles[st][bt]

            cursor = [0, 0]
            for b in range(NB):
                ap = pa.tile([P, ndout], f32, tag="agg")
                first = True
                if not final:
                    for st in (0, 1):
                        for _ in range(int(cnt[b, st])):
                            ci = cursor[st]
                            cursor[st] += 1
                            g, col = ci // GCH, ci % GCH
                            mt = ensure_group(st, g)
                            sl, g0 = ensure_batch(st, g // SB)
                            scol = (g - g0) * GCH + col
                            nc.tensor.matmul(
                                ap[:],
                                lhsT=sl[:, scol * P:(scol + 1) * P],
                                rhs=mt[:, col * P:(col + 1) * P],
                                start=first, stop=False,
                            )
                            first = False
                lhs_b = ones[:] if final else recipd[0:1, b * P:(b + 1) * P]
                if final:
                    nc.tensor.matmul(ap[:], lhsT=hTcur[:, b * P:(b + 1) * P],
                                     rhs=Wm[:], start=first, stop=False)
                    first = False
                nc.tensor.matmul(ap[:], lhsT=lhs_b, rhs=bm[:],
                                 start=first, stop=True)
                if final:
                    ot = opool.tile([P, DOUT_], f32, tag="o")
                    nc.scalar.activation(ot[:], ap[:],
                                         mybir.ActivationFunctionType.Copy)
                    nc.sync.dma_start(out_d[b * P:(b + 1) * P, :], ot[:])
                else:
                    hn = zpool.tile([P, P], f32, tag="hn")
                    nc.scalar.activation(hn[:], ap[:],
                                         mybir.ActivationFunctionType.Relu,
                                         scale=dinv_sb[:, b:b + 1])
                    tp = pt.tile([P, P], f32, tag="tp")
                    nc.tensor.transpose(out=tp[:], in_=hn[:], identity=ident[:])
                    nc.scalar.activation(hTnext[:, b * P:(b + 1) * P], tp[:],
                                         mybir.ActivationFunctionType.Copy)

        for layer in range(3):
            hTcur = hT[layer % 2]
            hTnext = hT[(layer + 1) % 2]
            slab_t = slab[layer % 2]
            table_t = table[layer % 2]
            for b in range(NB):
                zp = pz.tile([P, P], f32, tag="z")
                nc.tensor.matmul(zp[:], lhsT=hTcur[:, b * P:(b + 1) * P],
                                 rhs=w_sb[layer][:], start=True, stop=True)
                zt = zpool.tile([P, P], f16, tag="zt")
                nc.scalar.activation(zt[:], zp[:],
                                     mybir.ActivationFunctionType.Copy,
                                     scale=dinv_sb[:, b:b + 1])
                nc.sync.dma_start(slab_t[b * P:(b + 1) * P, :], zt[:])
            nc.gpsimd.collective_compute(
                "AllGather", mybir.AluOpType.bypass, replica_groups=rg,
                ins=[slab_t.opt()], outs=[table_t.opt()],
            )
            agg_phase(layer, hTcur, hTnext, table_t)
        agg_phase(3, hT[1], None, None, final=True)

    nc.compile()
    return nc


_CACHE = {}


def _get_compiled(edge_index):
    key = hash(np.asarray(edge_index, np.int64).tobytes())
    if key not in _CACHE:
        pp = _preprocess(edge_index, N_NODES, N_CORES, HALF)
        nc = _build(pp, DOUT, N_CORES)
        _CACHE[key] = (pp, nc)
    return _CACHE[key]


_LAST_RUN = {}


def kernel(x, edge_index, W1, b1, W2, b2, W3, b3, Wl, bl):
    x = np.asarray(x, np.float32)
    pp, nc = _get_compiled(edge_index)
    maps = _host_tensors(pp, x, (W1, b1, W2, b2, W3, b3, Wl, bl))

    from concourse.bass_utils import run_bass_kernel_spmd
    res = run_bass_kernel_spmd(nc, maps, core_ids=list(range(N_CORES)))
    LOCAL = pp["LOCAL"]
    out = np.concatenate(
        [np.asarray(res.results[c]["out"])[:LOCAL] for c in range(N_CORES)])
    _LAST_RUN["nc"] = nc
    _LAST_RUN["maps"] = maps
    return out


def _install_ntff_hook():
    """The agent image's antenv lacks axon_hooks; recreate it from the boot
    helper so run_bass_kernel_spmd(trace=True) can capture NTFF profiles."""
    import types
    if "antenv.axon_hooks" in sys.modules:
        return
    mod = types.ModuleType("antenv.axon_hooks")
    _state = {}
    mod.set_axon_ntff_profile_hook = lambda h: _state.__setitem__("h", h)
    mod.get_axon_ntff_profile_hook = lambda: _state.get("h")
    sys.modules["antenv.axon_hooks"] = mod
    import antenv
    antenv.axon_hooks = mod
    from trn_agent_boot.trn_boot import _ntff_profile_via_ctypes
    mod.set_axon_ntff_profile_hook(
        _ntff_profile_via_ctypes("/opt/axon/libaxon_pjrt.so"))


def profile_exec_ns():
    """Re-run the last kernel invocation with NTFF tracing; return exec ns."""
    if "nc" not in _LAST_RUN:
        return None
    _install_ntff_hook()
    from concourse.bass_utils import run_bass_kernel_spmd
    res = run_bass_kernel_spmd(
        _LAST_RUN["nc"], _LAST_RUN["maps"],
        core_ids=list(range(N_CORES)), trace=True,
    )
    _LAST_RUN["trace_res"] = res
    return res.exec_time_ns


# revision 4
# speedup vs baseline: 1.7643x; 1.4920x over previous
"""3-layer GCN + linear head on 8 TRN2 NeuronCores (Bass/Tile, SPMD).

Self-contained: hardcodes N=50000, E=600000, D=128, DOUT=32, 8 cores.

Math (matches the reference):
    src,dst + self-loops; deg = in-degree; dinv = rsqrt(deg)
    norm_e = dinv[src]*dinv[dst]
    layer(h): agg[d] = sum_e norm_e (hW)[src_e]; relu(agg+b)
    out = h3 @ Wl + bl

Device mapping: nodes sharded into 8 contiguous slabs (graph parallel).
Per layer: local Z~ = dinv ⊙ (H @ W) matmul -> AllGather the node table ->
bulk dma_gather of source rows (edges sorted by destination) -> segment-sum
via one-hot selection matmuls (sel built on DVE by iota compare)
accumulating per-128-destination-block PSUM -> relu + dinv scale (bias
folded in as a rank-1 matmul) -> PE transpose feeds the next layer's lhsT.

The node table is split in two halves (blocks 0..23 / 24..48 of each slab)
with separate AllGathers; each half-table has < 32768 rows so int16 gather
indices address it directly, and the second AllGather overlaps with the
next layer's first-half gathers. The next layer's Z~ matmul for block b is
emitted right after block b's aggregation closes, so each AllGather starts
as soon as its half-slab is ready — collectives run concurrently with the
tail of the previous aggregation phase.
"""
import sys
sys.path.insert(0, '/opt/trn_rl_repo')
import numpy as np

import concourse.bass as bass
import concourse.tile as tile
import concourse.mybir as mybir
from concourse import bacc
from concourse.library_config import mlp as mlp_lib

P = 128
GROUP = 1024      # edges per dma_gather (SWDGE ring carveout = 1024 desc)
GCH = GROUP // P
SB = 4            # gather groups per sel/idx batch
NQ = 4            # SWDGE queues

N_NODES = 50000
N_CORES = 8
DIN = 128
DOUT = 32


def _preprocess(edge_index, N, C):
    LOCAL = N // C
    NB = (LOCAL + P - 1) // P
    PADL = NB * P
    H0B = NB // 2
    H1B = NB - H0B
    H0R, H1R = H0B * P, H1B * P          # per-core rows per half
    TOT0, TOT1 = C * H0R, C * H1R        # table rows per half
    assert TOT0 <= 32768 and TOT1 <= 32768

    src = np.asarray(edge_index[0], dtype=np.int64)
    dst = np.asarray(edge_index[1], dtype=np.int64)
    loops = np.arange(N, dtype=np.int64)
    src = np.concatenate([src, loops])
    dst = np.concatenate([dst, loops])

    deg = np.bincount(dst, minlength=N).astype(np.float64)
    dinv = (1.0 / np.sqrt(deg)).astype(np.float32)
    sdeg = np.sqrt(deg).astype(np.float32)

    # source stream (which half-table) + id within that half-table
    score = src // LOCAL
    slocal = src % LOCAL
    s_st = (slocal >= H0R).astype(np.int64)
    sid = np.where(s_st == 0, score * H0R + slocal,
                   score * H1R + (slocal - H0R))

    core = dst // LOCAL
    ldst = dst % LOCAL

    per = [[[None, None] for _ in range(NB)] for _ in range(C)]
    for c in range(C):
        m = core == c
        cs, cl, cst = sid[m], ldst[m], s_st[m]
        order = np.argsort(cl, kind="stable")
        cs, cl, cst = cs[order], cl[order], cst[order]
        blk = cl // P
        dl = cl % P
        for b in range(NB):
            bm = blk == b
            for st in (0, 1):
                sm = bm & (cst == st)
                per[c][b][st] = (cs[sm], dl[sm])

    cnt = np.zeros((NB, 2), dtype=np.int64)
    for b in range(NB):
        for st in (0, 1):
            mx = max(len(per[c][b][st][0]) for c in range(C))
            cnt[b, st] = (mx + P - 1) // P

    n_chunks = [int(cnt[:, st].sum()) for st in (0, 1)]
    n_chunks_pad = [((n + GCH - 1) // GCH) * GCH if n else 0 for n in n_chunks]

    def wrap(idx_flat):
        g = len(idx_flat) // GROUP
        w = idx_flat.reshape(g, GROUP // 16, 16)
        w = np.transpose(w, (0, 2, 1))
        return np.tile(w, (1, 8, 1)).astype(np.int16)

    def dlocw(dl_flat):
        g = len(dl_flat) // GROUP
        d = dl_flat.reshape(g, GCH, P)
        return np.transpose(d, (0, 2, 1)).astype(np.float32)

    idx_w, dloc_w = [], []
    for c in range(C):
        sidx = [[], []]
        sdl = [[], []]
        for b in range(NB):
            for st in (0, 1):
                want = cnt[b, st] * P
                ii, dd = per[c][b][st]
                padn = want - len(ii)
                sidx[st].append(np.concatenate([ii, np.zeros(padn, np.int64)]))
                sdl[st].append(np.concatenate([dd, -np.ones(padn, np.int64)]))
        iw, dw = [], []
        for st in (0, 1):
            arr_i = np.concatenate(sidx[st]) if sidx[st] else np.zeros(0, np.int64)
            arr_d = np.concatenate(sdl[st]) if sdl[st] else np.zeros(0, np.int64)
            tail = n_chunks_pad[st] * P - len(arr_i)
            arr_i = np.concatenate([arr_i, np.zeros(tail, np.int64)])
            arr_d = np.concatenate([arr_d, -np.ones(tail, np.int64)])
            iw.append(wrap(arr_i))
            dw.append(dlocw(arr_d))
        idx_w.append(iw)
        dloc_w.append(dw)

    return dict(
        LOCAL=LOCAL, NB=NB, PADL=PADL, C=C,
        H0B=H0B, H1B=H1B, TOT0=TOT0, TOT1=TOT1,
        cnt=cnt, n_chunks_pad=n_chunks_pad,
        idx_w=idx_w, dloc_w=dloc_w, dinv=dinv, sdeg=sdeg,
    )


def _host_tensors(pp, x, weights):
    C, LOCAL, PADL, NB = pp["C"], pp["LOCAL"], pp["PADL"], pp["NB"]
    W1, b1, W2, b2, W3, b3, Wl, bl = weights
    iota = np.tile(np.arange(P, dtype=np.float32), (P, 1))
    ident = np.eye(P, dtype=np.float32)
    ones = np.ones((1, P), np.float32)
    maps = []
    for c in range(C):
        xs = np.zeros((PADL, P), np.float32)
        xs[:LOCAL] = x[c * LOCAL:(c + 1) * LOCAL]
        dvl = np.zeros(PADL, np.float32)
        dvl[:LOCAL] = pp["dinv"][c * LOCAL:(c + 1) * LOCAL]
        dv = np.ascontiguousarray(dvl.reshape(NB, P).T)
        rd = np.zeros((1, PADL), np.float32)
        rd[0, :LOCAL] = pp["sdeg"][c * LOCAL:(c + 1) * LOCAL]
        m = {
            "xt": np.ascontiguousarray(xs.T),
            "w1": np.ascontiguousarray(W1, np.float32),
            "w2": np.ascontiguousarray(W2, np.float32),
            "w3": np.ascontiguousarray(W3, np.float32),
            "wl": np.ascontiguousarray(Wl, np.float32),
            "b1": np.asarray(b1, np.float32).reshape(1, -1),
            "b2": np.asarray(b2, np.float32).reshape(1, -1),
            "b3": np.asarray(b3, np.float32).reshape(1, -1),
            "bl": np.asarray(bl, np.float32).reshape(1, -1),
            "dinv_sb": dv, "recipd": rd,
            "iota": iota, "ident": ident, "ones": ones,
        }
        for st in (0, 1):
            if pp["n_chunks_pad"][st]:
                m[f"idx{st}"] = pp["idx_w"][c][st]
                m[f"dloc{st}"] = pp["dloc_w"][c][st]
        maps.append(m)
    return maps


def _build(pp, DOUT_, n_cores):
    NB, PADL = pp["NB"], pp["PADL"]
    H0B, H1B, TOT0, TOT1 = pp["H0B"], pp["H1B"], pp["TOT0"], pp["TOT1"]
    cnt, n_chunks_pad = pp["cnt"], pp["n_chunks_pad"]
    f32 = mybir.dt.float32
    f16 = mybir.dt.float16

    nc = bacc.Bacc("TRN2", target_bir_lowering=False, debug=False,
                   num_devices=n_cores, num_swdge_queues=NQ)

    xt = nc.dram_tensor("xt", [P, PADL], f32, kind="ExternalInput")
    w = [nc.dram_tensor(f"w{i+1}", [P, P], f32, kind="ExternalInput") for i in range(3)]
    wl = nc.dram_tensor("wl", [P, DOUT_], f32, kind="ExternalInput")
    bias = [nc.dram_tensor(f"b{i+1}", [1, P], f32, kind="ExternalInput") for i in range(3)]
    bl = nc.dram_tensor("bl", [1, DOUT_], f32, kind="ExternalInput")
    dinv_sb_d = nc.dram_tensor("dinv_sb", [P, NB], f32, kind="ExternalInput")
    recipd_d = nc.dram_tensor("recipd", [1, PADL], f32, kind="ExternalInput")
    iota_d = nc.dram_tensor("iota", [P, P], f32, kind="ExternalInput")
    ident_d = nc.dram_tensor("ident", [P, P], f32, kind="ExternalInput")
    ones_d = nc.dram_tensor("ones", [1, P], f32, kind="ExternalInput")
    idx_d, dloc_d = [None, None], [None, None]
    for st in (0, 1):
        g = n_chunks_pad[st] // GCH
        if g:
            idx_d[st] = nc.dram_tensor(f"idx{st}", [g, P, GROUP // 16],
                                       mybir.dt.int16, kind="ExternalInput")
            dloc_d[st] = nc.dram_tensor(f"dloc{st}", [g, P, GCH], f32,
                                        kind="ExternalInput")
    out_d = nc.dram_tensor("out", [PADL, DOUT_], f32, kind="ExternalOutput")

    rg = [list(range(n_cores))]

    from contextlib import ExitStack
    with tile.TileContext(nc) as tc, ExitStack() as ctx:
        dram = ctx.enter_context(tc.tile_pool(name="dram", bufs=1, space="DRAM"))
        cpool = ctx.enter_context(tc.tile_pool(name="consts", bufs=1))
        hpool = ctx.enter_context(tc.tile_pool(name="ht", bufs=1))
        mpool = ctx.enter_context(tc.tile_pool(name="msg", bufs=16))
        spool = ctx.enter_context(tc.tile_pool(name="sel", bufs=3))
        dpool = ctx.enter_context(tc.tile_pool(name="dloc", bufs=3))
        ipool = ctx.enter_context(tc.tile_pool(name="idx", bufs=3))
        zpool = ctx.enter_context(tc.tile_pool(name="zt", bufs=3))
        opool = ctx.enter_context(tc.tile_pool(name="outs", bufs=3))
        pz = ctx.enter_context(tc.tile_pool(name="pz", bufs=2, space="PSUM"))
        pa = ctx.enter_context(tc.tile_pool(name="pa", bufs=2, space="PSUM"))
        pt = ctx.enter_context(tc.tile_pool(name="pt", bufs=2, space="PSUM"))

        nc.gpsimd.load_library(mlp_lib)

        def const(dram_t, shape):
            t = cpool.tile(shape, f32, name=dram_t.name + "_sb")
            nc.sync.dma_start(t[:], dram_t[:])
            return t
        w_sb = [const(w[i], [P, P]) for i in range(3)]
        wl_sb = const(wl, [P, DOUT_])
        b_sb = [const(bias[i], [1, P]) for i in range(3)]
        bl_sb = const(bl, [1, DOUT_])
        dinv_sb = const(dinv_sb_d, [P, NB])
        recipd = const(recipd_d, [1, PADL])
        iota = const(iota_d, [P, P])
        ident = const(ident_d, [P, P])
        ones = const(ones_d, [1, P])

        hT = [hpool.tile([P, PADL], f32, name=f"hT{i}") for i in range(2)]
        nc.sync.dma_start(hT[0][:], xt[:])

        slab = [[dram.tile([H0B * P, P], f16, name=f"slab0_{i}"),
                 dram.tile([H1B * P, P], f16, name=f"slab1_{i}")]
                for i in range(2)]
        table = [[tc.tile([TOT0, P], f16, space="DRAM", addr_space="Shared",
                          name=f"table0_{i}")[0],
                  tc.tile([TOT1, P], f16, space="DRAM", addr_space="Shared",
                          name=f"table1_{i}")[0]]
                 for i in range(2)]

        gq = [0]

        def z_block(layer, hTsrc, b):
            """Emit Z~ matmul for block b of `layer`, write to layer's slab."""
            buf = layer % 2
            zp = pz.tile([P, P], f32, tag="z")
            nc.tensor.matmul(zp[:], lhsT=hTsrc[:, b * P:(b + 1) * P],
                             rhs=w_sb[layer][:], start=True, stop=True)
            zt = zpool.tile([P, P], f16, tag="zt")
            nc.scalar.activation(zt[:], zp[:],
                                 mybir.ActivationFunctionType.Copy,
                                 scale=dinv_sb[:, b:b + 1])
            if b < H0B:
                nc.sync.dma_start(slab[buf][0][b * P:(b + 1) * P, :], zt[:])
            else:
                bb = b - H0B
                nc.sync.dma_start(slab[buf][1][bb * P:(bb + 1) * P, :], zt[:])

        def ag(layer, half):
            buf = layer % 2
            nc.gpsimd.collective_compute(
                "AllGather", mybir.AluOpType.bypass, replica_groups=rg,
                ins=[slab[buf][half].opt()], outs=[table[buf][half].opt()],
            )

        def agg_phase(layer, hTcur, hTnext, final=False):
            """Aggregation for `layer`; also emits layer+1's Z~/AG per block."""
            bm = bl_sb if final else b_sb[layer]
            ndout = DOUT_ if final else P
            buf = layer % 2
            batch_tiles = [{}, {}]

            def ensure_batch(st, bt):
                if bt in batch_tiles[st]:
                    return batch_tiles[st][bt]
                g0 = bt * SB
                ng = min(SB, n_chunks_pad[st] // GCH - g0)
                nchk = ng * GCH
                it = ipool.tile([P, ng * (GROUP // 16)], mybir.dt.int16, tag="idx")
                nc.sync.dma_start(
                    it[:].rearrange("p (g m) -> p g m", m=GROUP // 16),
                    idx_d[st][g0:g0 + ng].rearrange("g p m -> p g m"),
                )
                dlt = dpool.tile([P, nchk], f32, tag="dloc")
                nc.sync.dma_start(
                    dlt[:].rearrange("p (g m) -> p g m", m=GCH),
                    dloc_d[st][g0:g0 + ng].rearrange("g p m -> p g m"),
                )
                st_t = spool.tile([P, nchk * P], f16, tag="sel")
                nc.vector.tensor_tensor(
                    out=st_t[:].rearrange("p (m j) -> p m j", j=P),
                    in0=dlt[:].to_broadcast([P, nchk, P]),
                    in1=iota[:].unsqueeze(1).to_broadcast([P, nchk, P]),
                    op=mybir.AluOpType.is_equal,
                )
                batch_tiles[st][bt] = (it, st_t, g0)
                return batch_tiles[st][bt]

            msg_tiles = [{}, {}]

            def ensure_group(st, g):
                if g in msg_tiles[st]:
                    return msg_tiles[st][g]
                it, _, g0 = ensure_batch(st, g // SB)
                mt = mpool.tile([P, GROUP], f16, tag="msg")
                iw = GROUP // 16
                nc.gpsimd.dma_gather(
                    out_ap=mt[:].rearrange("p (c e) -> p c e", e=P),
                    in_ap=table[buf][st][:],
                    idxs_ap=it[:, (g - g0) * iw:(g - g0 + 1) * iw],
                    num_idxs=GROUP,
                    num_idxs_reg=GROUP,
                    elem_size=P,
                    queue_num=gq[0] % NQ,
                )
                gq[0] += 1
                msg_tiles[st][g] = mt
                return mt

            cursor = [0, 0]
            for b in range(NB):
                ap = pa.tile([P, ndout], f32, tag="agg")
                first = True
                if not final:
                    for st in (0, 1):
                        for _ in range(int(cnt[b, st])):
                            ci = cursor[st]
                            cursor[st] += 1
                            g, col = ci // GCH, ci % GCH
                            mt = ensure_group(st, g)
                            _, sl, g0 = ensure_batch(st, g // SB)
                            scol = (g - g0) * GCH + col
                            nc.tensor.matmul(
                                ap[:],
                                lhsT=sl[:, scol * P:(scol + 1) * P],
                                rhs=mt[:, col * P:(col + 1) * P],
                                start=first, stop=False,
                            )
                            first = False
                lhs_b = ones[:] if final else recipd[0:1, b * P:(b + 1) * P]
                if final:
                    nc.tensor.matmul(ap[:], lhsT=hTcur[:, b * P:(b + 1) * P],
                                     rhs=wl_sb[:], start=first, stop=False)
                    first = False
                nc.tensor.matmul(ap[:], lhsT=lhs_b, rhs=bm[:],
                                 start=first, stop=True)
                if final:
                    ot = opool.tile([P, DOUT_], f32, tag="o")
                    nc.scalar.activation(ot[:], ap[:],
                                         mybir.ActivationFunctionType.Copy)
                    nc.sync.dma_start(out_d[b * P:(b + 1) * P, :], ot[:])
                else:
                    hn = zpool.tile([P, P], f32, tag="hn")
                    nc.scalar.activation(hn[:], ap[:],
                                         mybir.ActivationFunctionType.Relu,
                                         scale=dinv_sb[:, b:b + 1])
                    tp = pt.tile([P, P], f32, tag="tp")
                    nc.tensor.transpose(out=tp[:], in_=hn[:], identity=ident[:])
                    nc.scalar.activation(hTnext[:, b * P:(b + 1) * P], tp[:],
                                         mybir.ActivationFunctionType.Copy)
                    if layer < 2:
                        # next layer's Z~ for this block; fire AGs when a
                        # half-slab completes
                        z_block(layer + 1, hTnext, b)
                        if b == H0B - 1:
                            ag(layer + 1, 0)
                        elif b == NB - 1:
                            ag(layer + 1, 1)

        # prologue: layer 0 Z~ from x
        for b in range(NB):
            z_block(0, hT[0], b)
            if b == H0B - 1:
                ag(0, 0)
            elif b == NB - 1:
                ag(0, 1)

        for layer in range(3):
            agg_phase(layer, hT[layer % 2], hT[(layer + 1) % 2])
        agg_phase(3, hT[1], None, final=True)

    nc.compile()
    return nc


_CACHE = {}


def _get_compiled(edge_index):
    key = hash(np.asarray(edge_index, np.int64).tobytes())
    if key not in _CACHE:
        pp = _preprocess(edge_index, N_NODES, N_CORES)
        nc = _build(pp, DOUT, N_CORES)
        _CACHE[key] = (pp, nc)
    return _CACHE[key]


_LAST_RUN = {}


def kernel(x, edge_index, W1, b1, W2, b2, W3, b3, Wl, bl):
    x = np.asarray(x, np.float32)
    pp, nc = _get_compiled(edge_index)
    maps = _host_tensors(pp, x, (W1, b1, W2, b2, W3, b3, Wl, bl))

    from concourse.bass_utils import run_bass_kernel_spmd
    res = run_bass_kernel_spmd(nc, maps, core_ids=list(range(N_CORES)))
    LOCAL = pp["LOCAL"]
    out = np.concatenate(
        [np.asarray(res.results[c]["out"])[:LOCAL] for c in range(N_CORES)])
    _LAST_RUN["nc"] = nc
    _LAST_RUN["maps"] = maps
    return out


def _install_ntff_hook():
    """The agent image's antenv lacks axon_hooks; recreate it from the boot
    helper so run_bass_kernel_spmd(trace=True) can capture NTFF profiles."""
    import types
    if "antenv.axon_hooks" in sys.modules:
        return
    mod = types.ModuleType("antenv.axon_hooks")
    _state = {}
    mod.set_axon_ntff_profile_hook = lambda h: _state.__setitem__("h", h)
    mod.get_axon_ntff_profile_hook = lambda: _state.get("h")
    sys.modules["antenv.axon_hooks"] = mod
    import antenv
    antenv.axon_hooks = mod
    from trn_agent_boot.trn_boot import _ntff_profile_via_ctypes
    mod.set_axon_ntff_profile_hook(
        _ntff_profile_via_ctypes("/opt/axon/libaxon_pjrt.so"))


def profile_exec_ns():
    """Re-run the last kernel invocation with NTFF tracing; return exec ns."""
    if "nc" not in _LAST_RUN:
        return None
    _install_ntff_hook()
    from concourse.bass_utils import run_bass_kernel_spmd
    res = run_bass_kernel_spmd(
        _LAST_RUN["nc"], _LAST_RUN["maps"],
        core_ids=list(range(N_CORES)), trace=True,
    )
    _LAST_RUN["trace_res"] = res
    return res.exec_time_ns


# revision 5
# speedup vs baseline: 1.7797x; 1.0087x over previous
"""3-layer GCN + linear head on 8 TRN2 NeuronCores (Bass/Tile, SPMD).

Self-contained: hardcodes N=50000, E=600000, D=128, DOUT=32, 8 cores.

Math (matches the reference):
    src,dst + self-loops; deg = in-degree; dinv = rsqrt(deg)
    norm_e = dinv[src]*dinv[dst]
    layer(h): agg[d] = sum_e norm_e (hW)[src_e]; relu(agg+b)
    out = h3 @ Wl + bl

Device mapping: nodes sharded into 8 contiguous slabs (graph parallel).
Per layer: local Z~ = dinv ⊙ (H @ W) matmul -> AllGather the node table ->
bulk dma_gather of source rows (edges sorted by destination) -> segment-sum
via one-hot selection matmuls (sel built on DVE by iota compare)
accumulating per-128-destination-block PSUM -> relu + dinv scale (bias
folded in as a rank-1 matmul) -> PE transpose feeds the next layer's lhsT.

The node table is split in two halves (blocks 0..23 / 24..48 of each slab)
with separate AllGathers; each half-table has < 32768 rows so int16 gather
indices address it directly, and the second AllGather overlaps with the
next layer's first-half gathers. The next layer's Z~ matmul for block b is
emitted right after block b's aggregation closes, so each AllGather starts
as soon as its half-slab is ready — collectives run concurrently with the
tail of the previous aggregation phase.
"""
import sys
sys.path.insert(0, '/opt/trn_rl_repo')
import numpy as np

import concourse.bass as bass
import concourse.tile as tile
import concourse.mybir as mybir
from concourse import bacc
from concourse.library_config import mlp as mlp_lib

P = 128
GROUP = 1024      # edges per dma_gather (SWDGE ring carveout = 1024 desc)
GCH = GROUP // P
SB = 4            # gather groups per sel/idx batch
NQ = 4            # SWDGE queues

N_NODES = 50000
N_CORES = 8
DIN = 128
DOUT = 32


def _preprocess(edge_index, N, C):
    LOCAL = N // C
    NB = (LOCAL + P - 1) // P
    PADL = NB * P
    H0B = NB // 2
    H1B = NB - H0B
    H0R, H1R = H0B * P, H1B * P          # per-core rows per half
    TOT0, TOT1 = C * H0R, C * H1R        # table rows per half
    assert TOT0 <= 32768 and TOT1 <= 32768

    src = np.asarray(edge_index[0], dtype=np.int64)
    dst = np.asarray(edge_index[1], dtype=np.int64)
    loops = np.arange(N, dtype=np.int64)
    src = np.concatenate([src, loops])
    dst = np.concatenate([dst, loops])

    deg = np.bincount(dst, minlength=N).astype(np.float64)
    dinv = (1.0 / np.sqrt(deg)).astype(np.float32)
    sdeg = np.sqrt(deg).astype(np.float32)

    # source stream (which half-table) + id within that half-table
    score = src // LOCAL
    slocal = src % LOCAL
    s_st = (slocal >= H0R).astype(np.int64)
    sid = np.where(s_st == 0, score * H0R + slocal,
                   score * H1R + (slocal - H0R))

    core = dst // LOCAL
    ldst = dst % LOCAL

    per = [[[None, None] for _ in range(NB)] for _ in range(C)]
    for c in range(C):
        m = core == c
        cs, cl, cst = sid[m], ldst[m], s_st[m]
        order = np.argsort(cl, kind="stable")
        cs, cl, cst = cs[order], cl[order], cst[order]
        blk = cl // P
        dl = cl % P
        for b in range(NB):
            bm = blk == b
            for st in (0, 1):
                sm = bm & (cst == st)
                per[c][b][st] = (cs[sm], dl[sm])

    cnt = np.zeros((NB, 2), dtype=np.int64)
    for b in range(NB):
        for st in (0, 1):
            mx = max(len(per[c][b][st][0]) for c in range(C))
            cnt[b, st] = (mx + P - 1) // P

    n_chunks = [int(cnt[:, st].sum()) for st in (0, 1)]
    n_chunks_pad = [((n + GCH - 1) // GCH) * GCH if n else 0 for n in n_chunks]

    def wrap(idx_flat):
        g = len(idx_flat) // GROUP
        w = idx_flat.reshape(g, GROUP // 16, 16)
        w = np.transpose(w, (0, 2, 1))
        return np.tile(w, (1, 8, 1)).astype(np.int16)

    def dlocw(dl_flat):
        g = len(dl_flat) // GROUP
        d = dl_flat.reshape(g, GCH, P)
        return np.transpose(d, (0, 2, 1)).astype(np.float32)

    idx_w, dloc_w = [], []
    for c in range(C):
        sidx = [[], []]
        sdl = [[], []]
        for b in range(NB):
            for st in (0, 1):
                want = cnt[b, st] * P
                ii, dd = per[c][b][st]
                padn = want - len(ii)
                sidx[st].append(np.concatenate([ii, np.zeros(padn, np.int64)]))
                sdl[st].append(np.concatenate([dd, -np.ones(padn, np.int64)]))
        iw, dw = [], []
        for st in (0, 1):
            arr_i = np.concatenate(sidx[st]) if sidx[st] else np.zeros(0, np.int64)
            arr_d = np.concatenate(sdl[st]) if sdl[st] else np.zeros(0, np.int64)
            tail = n_chunks_pad[st] * P - len(arr_i)
            arr_i = np.concatenate([arr_i, np.zeros(tail, np.int64)])
            arr_d = np.concatenate([arr_d, -np.ones(tail, np.int64)])
            iw.append(wrap(arr_i))
            dw.append(dlocw(arr_d))
        idx_w.append(iw)
        dloc_w.append(dw)

    return dict(
        LOCAL=LOCAL, NB=NB, PADL=PADL, C=C,
        H0B=H0B, H1B=H1B, TOT0=TOT0, TOT1=TOT1,
        cnt=cnt, n_chunks_pad=n_chunks_pad,
        idx_w=idx_w, dloc_w=dloc_w, dinv=dinv, sdeg=sdeg,
    )


def _host_tensors(pp, x, weights):
    C, LOCAL, PADL, NB = pp["C"], pp["LOCAL"], pp["PADL"], pp["NB"]
    W1, b1, W2, b2, W3, b3, Wl, bl = weights
    iota = np.tile(np.arange(P, dtype=np.float32), (P, 1))
    ident = np.eye(P, dtype=np.float32)
    ones = np.ones((1, P), np.float32)
    maps = []
    for c in range(C):
        xs = np.zeros((PADL, P), np.float32)
        xs[:LOCAL] = x[c * LOCAL:(c + 1) * LOCAL]
        dvl = np.zeros(PADL, np.float32)
        dvl[:LOCAL] = pp["dinv"][c * LOCAL:(c + 1) * LOCAL]
        dv = np.ascontiguousarray(dvl.reshape(NB, P).T)
        rd = np.zeros((1, PADL), np.float32)
        rd[0, :LOCAL] = pp["sdeg"][c * LOCAL:(c + 1) * LOCAL]
        m = {
            "xt": np.ascontiguousarray(xs.T),
            "w1": np.ascontiguousarray(W1, np.float32),
            "w2": np.ascontiguousarray(W2, np.float32),
            "w3": np.ascontiguousarray(W3, np.float32),
            "wl": np.ascontiguousarray(Wl, np.float32),
            "b1": np.asarray(b1, np.float32).reshape(1, -1),
            "b2": np.asarray(b2, np.float32).reshape(1, -1),
            "b3": np.asarray(b3, np.float32).reshape(1, -1),
            "bl": np.asarray(bl, np.float32).reshape(1, -1),
            "dinv_sb": dv, "recipd": rd,
            "iota": iota, "ident": ident, "ones": ones,
        }
        for st in (0, 1):
            if pp["n_chunks_pad"][st]:
                m[f"idx{st}"] = pp["idx_w"][c][st]
                m[f"dloc{st}"] = pp["dloc_w"][c][st]
        maps.append(m)
    return maps


def _build(pp, DOUT_, n_cores):
    NB, PADL = pp["NB"], pp["PADL"]
    H0B, H1B, TOT0, TOT1 = pp["H0B"], pp["H1B"], pp["TOT0"], pp["TOT1"]
    cnt, n_chunks_pad = pp["cnt"], pp["n_chunks_pad"]
    f32 = mybir.dt.float32
    f16 = mybir.dt.float16

    nc = bacc.Bacc("TRN2", target_bir_lowering=False, debug=False,
                   num_devices=n_cores, num_swdge_queues=NQ)

    xt = nc.dram_tensor("xt", [P, PADL], f32, kind="ExternalInput")
    w = [nc.dram_tensor(f"w{i+1}", [P, P], f32, kind="ExternalInput") for i in range(3)]
    wl = nc.dram_tensor("wl", [P, DOUT_], f32, kind="ExternalInput")
    bias = [nc.dram_tensor(f"b{i+1}", [1, P], f32, kind="ExternalInput") for i in range(3)]
    bl = nc.dram_tensor("bl", [1, DOUT_], f32, kind="ExternalInput")
    dinv_sb_d = nc.dram_tensor("dinv_sb", [P, NB], f32, kind="ExternalInput")
    recipd_d = nc.dram_tensor("recipd", [1, PADL], f32, kind="ExternalInput")
    iota_d = nc.dram_tensor("iota", [P, P], f32, kind="ExternalInput")
    ident_d = nc.dram_tensor("ident", [P, P], f32, kind="ExternalInput")
    ones_d = nc.dram_tensor("ones", [1, P], f32, kind="ExternalInput")
    idx_d, dloc_d = [None, None], [None, None]
    for st in (0, 1):
        g = n_chunks_pad[st] // GCH
        if g:
            idx_d[st] = nc.dram_tensor(f"idx{st}", [g, P, GROUP // 16],
                                       mybir.dt.int16, kind="ExternalInput")
            dloc_d[st] = nc.dram_tensor(f"dloc{st}", [g, P, GCH], f32,
                                        kind="ExternalInput")
    out_d = nc.dram_tensor("out", [PADL, DOUT_], f32, kind="ExternalOutput")

    rg = [list(range(n_cores))]

    from contextlib import ExitStack
    with tile.TileContext(nc) as tc, ExitStack() as ctx:
        dram = ctx.enter_context(tc.tile_pool(name="dram", bufs=1, space="DRAM"))
        cpool = ctx.enter_context(tc.tile_pool(name="consts", bufs=1))
        hpool = ctx.enter_context(tc.tile_pool(name="ht", bufs=1))
        mpool = ctx.enter_context(tc.tile_pool(name="msg", bufs=22))
        spool = ctx.enter_context(tc.tile_pool(name="sel", bufs=4))
        dpool = ctx.enter_context(tc.tile_pool(name="dloc", bufs=3))
        ipool = ctx.enter_context(tc.tile_pool(name="idx", bufs=3))
        zpool = ctx.enter_context(tc.tile_pool(name="zt", bufs=3))
        opool = ctx.enter_context(tc.tile_pool(name="outs", bufs=3))
        pz = ctx.enter_context(tc.tile_pool(name="pz", bufs=2, space="PSUM"))
        pa = ctx.enter_context(tc.tile_pool(name="pa", bufs=3, space="PSUM"))
        pt = ctx.enter_context(tc.tile_pool(name="pt", bufs=2, space="PSUM"))

        nc.gpsimd.load_library(mlp_lib)

        def const(dram_t, shape):
            t = cpool.tile(shape, f32, name=dram_t.name + "_sb")
            nc.sync.dma_start(t[:], dram_t[:])
            return t
        w_sb = [const(w[i], [P, P]) for i in range(3)]
        wl_sb = const(wl, [P, DOUT_])
        b_sb = [const(bias[i], [1, P]) for i in range(3)]
        bl_sb = const(bl, [1, DOUT_])
        dinv_sb = const(dinv_sb_d, [P, NB])
        recipd = const(recipd_d, [1, PADL])
        iota = const(iota_d, [P, P])
        ident = const(ident_d, [P, P])
        ones = const(ones_d, [1, P])

        hT = [hpool.tile([P, PADL], f32, name=f"hT{i}") for i in range(2)]
        nc.sync.dma_start(hT[0][:], xt[:])

        slab = [[dram.tile([H0B * P, P], f16, name=f"slab0_{i}"),
                 dram.tile([H1B * P, P], f16, name=f"slab1_{i}")]
                for i in range(2)]
        table = [[tc.tile([TOT0, P], f16, space="DRAM", addr_space="Shared",
                          name=f"table0_{i}")[0],
                  tc.tile([TOT1, P], f16, space="DRAM", addr_space="Shared",
                          name=f"table1_{i}")[0]]
                 for i in range(2)]

        gq = [0]

        def z_block(layer, hTsrc, b):
            """Emit Z~ matmul for block b of `layer`, write to layer's slab."""
            buf = layer % 2
            zp = pz.tile([P, P], f32, tag="z")
            nc.tensor.matmul(zp[:], lhsT=hTsrc[:, b * P:(b + 1) * P],
                             rhs=w_sb[layer][:], start=True, stop=True)
            zt = zpool.tile([P, P], f16, tag="zt")
            nc.scalar.activation(zt[:], zp[:],
                                 mybir.ActivationFunctionType.Copy,
                                 scale=dinv_sb[:, b:b + 1])
            if b < H0B:
                nc.sync.dma_start(slab[buf][0][b * P:(b + 1) * P, :], zt[:])
            else:
                bb = b - H0B
                nc.sync.dma_start(slab[buf][1][bb * P:(bb + 1) * P, :], zt[:])

        def ag(layer, half):
            buf = layer % 2
            nc.gpsimd.collective_compute(
                "AllGather", mybir.AluOpType.bypass, replica_groups=rg,
                ins=[slab[buf][half].opt()], outs=[table[buf][half].opt()],
            )

        def agg_phase(layer, hTcur, hTnext, final=False):
            """Aggregation for `layer`; also emits layer+1's Z~/AG per block."""
            bm = bl_sb if final else b_sb[layer]
            ndout = DOUT_ if final else P
            buf = layer % 2
            batch_tiles = [{}, {}]

            def ensure_batch(st, bt):
                if bt in batch_tiles[st]:
                    return batch_tiles[st][bt]
                g0 = bt * SB
                ng = min(SB, n_chunks_pad[st] // GCH - g0)
                nchk = ng * GCH
                it = ipool.tile([P, ng * (GROUP // 16)], mybir.dt.int16, tag="idx")
                nc.sync.dma_start(
                    it[:].rearrange("p (g m) -> p g m", m=GROUP // 16),
                    idx_d[st][g0:g0 + ng].rearrange("g p m -> p g m"),
                )
                dlt = dpool.tile([P, nchk], f32, tag="dloc")
                nc.sync.dma_start(
                    dlt[:].rearrange("p (g m) -> p g m", m=GCH),
                    dloc_d[st][g0:g0 + ng].rearrange("g p m -> p g m"),
                )
                st_t = spool.tile([P, nchk * P], f16, tag="sel")
                nc.vector.tensor_tensor(
                    out=st_t[:].rearrange("p (m j) -> p m j", j=P),
                    in0=dlt[:].to_broadcast([P, nchk, P]),
                    in1=iota[:].unsqueeze(1).to_broadcast([P, nchk, P]),
                    op=mybir.AluOpType.is_equal,
                )
                batch_tiles[st][bt] = (it, st_t, g0)
                return batch_tiles[st][bt]

            msg_tiles = [{}, {}]

            def ensure_group(st, g):
                if g in msg_tiles[st]:
                    return msg_tiles[st][g]
                it, _, g0 = ensure_batch(st, g // SB)
                mt = mpool.tile([P, GROUP], f16, tag="msg")
                iw = GROUP // 16
                nc.gpsimd.dma_gather(
                    out_ap=mt[:].rearrange("p (c e) -> p c e", e=P),
                    in_ap=table[buf][st][:],
                    idxs_ap=it[:, (g - g0) * iw:(g - g0 + 1) * iw],
                    num_idxs=GROUP,
                    num_idxs_reg=GROUP,
                    elem_size=P,
                    queue_num=gq[0] % NQ,
                )
                gq[0] += 1
                msg_tiles[st][g] = mt
                return mt

            cursor = [0, 0]
            for b in range(NB):
                ap = pa.tile([P, ndout], f32, tag="agg")
                first = True
                if not final:
                    for st in (0, 1):
                        for _ in range(int(cnt[b, st])):
                            ci = cursor[st]
                            cursor[st] += 1
                            g, col = ci // GCH, ci % GCH
                            mt = ensure_group(st, g)
                            _, sl, g0 = ensure_batch(st, g // SB)
                            scol = (g - g0) * GCH + col
                            nc.tensor.matmul(
                                ap[:],
                                lhsT=sl[:, scol * P:(scol + 1) * P],
                                rhs=mt[:, col * P:(col + 1) * P],
                                start=first, stop=False,
                            )
                            first = False
                lhs_b = ones[:] if final else recipd[0:1, b * P:(b + 1) * P]
                if final:
                    nc.tensor.matmul(ap[:], lhsT=hTcur[:, b * P:(b + 1) * P],
                                     rhs=wl_sb[:], start=first, stop=False)
                    first = False
                nc.tensor.matmul(ap[:], lhsT=lhs_b, rhs=bm[:],
                                 start=first, stop=True)
                if final:
                    ot = opool.tile([P, DOUT_], f32, tag="o")
                    nc.scalar.activation(ot[:], ap[:],
                                         mybir.ActivationFunctionType.Copy)
                    nc.sync.dma_start(out_d[b * P:(b + 1) * P, :], ot[:])
                else:
                    hn = zpool.tile([P, P], f32, tag="hn")
                    nc.scalar.activation(hn[:], ap[:],
                                         mybir.ActivationFunctionType.Relu,
                                         scale=dinv_sb[:, b:b + 1])
                    tp = pt.tile([P, P], f32, tag="tp")
                    nc.tensor.transpose(out=tp[:], in_=hn[:], identity=ident[:])
                    nc.scalar.activation(hTnext[:, b * P:(b + 1) * P], tp[:],
                                         mybir.ActivationFunctionType.Copy)
                    if layer < 2:
                        # next layer's Z~ for this block; fire AGs when a
                        # half-slab completes
                        z_block(layer + 1, hTnext, b)
                        if b == H0B - 1:
                            ag(layer + 1, 0)
                        elif b == NB - 1:
                            ag(layer + 1, 1)

        # prologue: layer 0 Z~ from x
        for b in range(NB):
            z_block(0, hT[0], b)
            if b == H0B - 1:
                ag(0, 0)
            elif b == NB - 1:
                ag(0, 1)

        for layer in range(3):
            agg_phase(layer, hT[layer % 2], hT[(layer + 1) % 2])
        agg_phase(3, hT[1], None, final=True)

    nc.compile()
    return nc


_CACHE = {}


def _get_compiled(edge_index):
    key = hash(np.asarray(edge_index, np.int64).tobytes())
    if key not in _CACHE:
        pp = _preprocess(edge_index, N_NODES, N_CORES)
        nc = _build(pp, DOUT, N_CORES)
        _CACHE[key] = (pp, nc)
    return _CACHE[key]


_LAST_RUN = {}


def kernel(x, edge_index, W1, b1, W2, b2, W3, b3, Wl, bl):
    x = np.asarray(x, np.float32)
    pp, nc = _get_compiled(edge_index)
    maps = _host_tensors(pp, x, (W1, b1, W2, b2, W3, b3, Wl, bl))

    from concourse.bass_utils import run_bass_kernel_spmd
    res = run_bass_kernel_spmd(nc, maps, core_ids=list(range(N_CORES)))
    LOCAL = pp["LOCAL"]
    out = np.concatenate(
        [np.asarray(res.results[c]["out"])[:LOCAL] for c in range(N_CORES)])
    _LAST_RUN["nc"] = nc
    _LAST_RUN["maps"] = maps
    return out


def _install_ntff_hook():
    """The agent image's antenv lacks axon_hooks; recreate it from the boot
    helper so run_bass_kernel_spmd(trace=True) can capture NTFF profiles."""
    import types
    if "antenv.axon_hooks" in sys.modules:
        return
    mod = types.ModuleType("antenv.axon_hooks")
    _state = {}
    mod.set_axon_ntff_profile_hook = lambda h: _state.__setitem__("h", h)
    mod.get_axon_ntff_profile_hook = lambda: _state.get("h")
    sys.modules["antenv.axon_hooks"] = mod
    import antenv
    antenv.axon_hooks = mod
    from trn_agent_boot.trn_boot import _ntff_profile_via_ctypes
    mod.set_axon_ntff_profile_hook(
        _ntff_profile_via_ctypes("/opt/axon/libaxon_pjrt.so"))


def profile_exec_ns():
    """Re-run the last kernel invocation with NTFF tracing; return exec ns."""
    if "nc" not in _LAST_RUN:
        return None
    _install_ntff_hook()
    from concourse.bass_utils import run_bass_kernel_spmd
    res = run_bass_kernel_spmd(
        _LAST_RUN["nc"], _LAST_RUN["maps"],
        core_ids=list(range(N_CORES)), trace=True,
    )
    _LAST_RUN["trace_res"] = res
    return res.exec_time_ns


# revision 7
# speedup vs baseline: 1.9445x; 1.0926x over previous
"""3-layer GCN + linear head on 8 TRN2 NeuronCores (Bass/Tile, SPMD).

Self-contained: hardcodes N=50000, E=600000, D=128, DOUT=32, 8 cores.

Math (matches the reference):
    src,dst + self-loops; deg = in-degree; dinv = rsqrt(deg)
    norm_e = dinv[src]*dinv[dst]
    layer(h): agg[d] = sum_e norm_e (hW)[src_e]; relu(agg+b)
    out = h3 @ Wl + bl

Device mapping: nodes sharded into 8 contiguous slabs (graph parallel).
Per layer: local Z~ = dinv ⊙ (H @ W) matmul -> AllGather the node table ->
bulk dma_gather of source rows (edges sorted by destination) -> segment-sum
via one-hot selection matmuls (sel built on DVE by iota compare)
accumulating per-128-destination-block PSUM -> relu + dinv scale (bias
folded in as a rank-1 matmul) -> PE transpose feeds the next layer's lhsT.

The node table is split in two halves (blocks 0..23 / 24..48 of each slab)
with separate AllGathers; each half-table has < 32768 rows so int16 gather
indices address it directly, and the second AllGather overlaps with the
next layer's first-half gathers. The next layer's Z~ matmul for block b is
emitted right after block b's aggregation closes, so each AllGather starts
as soon as its half-slab is ready — collectives run concurrently with the
tail of the previous aggregation phase.
"""
import sys
sys.path.insert(0, '/opt/trn_rl_repo')
import numpy as np

import concourse.bass as bass
import concourse.tile as tile
import concourse.mybir as mybir
from concourse import bacc
from concourse.library_config import mlp as mlp_lib

P = 128
GROUP = 1024      # edges per dma_gather (SWDGE ring carveout = 1024 desc)
GCH = GROUP // P
SB = 4            # gather groups per sel/idx batch
NQ = 4            # SWDGE queues

N_NODES = 50000
N_CORES = 8
DIN = 128
DOUT = 32


def _preprocess(edge_index, N, C):
    LOCAL = N // C
    NB = (LOCAL + P - 1) // P
    PADL = NB * P
    H0B = NB // 2
    H1B = NB - H0B
    H0R, H1R = H0B * P, H1B * P          # per-core rows per half
    TOT0, TOT1 = C * H0R, C * H1R        # table rows per half
    assert TOT0 <= 32768 and TOT1 <= 32768

    src = np.asarray(edge_index[0], dtype=np.int64)
    dst = np.asarray(edge_index[1], dtype=np.int64)
    loops = np.arange(N, dtype=np.int64)
    src = np.concatenate([src, loops])
    dst = np.concatenate([dst, loops])

    deg = np.bincount(dst, minlength=N).astype(np.float64)
    dinv = (1.0 / np.sqrt(deg)).astype(np.float32)
    sdeg = np.sqrt(deg).astype(np.float32)

    # source stream (which half-table) + id within that half-table
    score = src // LOCAL
    slocal = src % LOCAL
    s_st = (slocal >= H0R).astype(np.int64)
    sid = np.where(s_st == 0, score * H0R + slocal,
                   score * H1R + (slocal - H0R))

    core = dst // LOCAL
    ldst = dst % LOCAL

    per = [[[None, None] for _ in range(NB)] for _ in range(C)]
    for c in range(C):
        m = core == c
        cs, cl, cst = sid[m], ldst[m], s_st[m]
        order = np.argsort(cl, kind="stable")
        cs, cl, cst = cs[order], cl[order], cst[order]
        blk = cl // P
        dl = cl % P
        for b in range(NB):
            bm = blk == b
            for st in (0, 1):
                sm = bm & (cst == st)
                per[c][b][st] = (cs[sm], dl[sm])

    cnt = np.zeros((NB, 2), dtype=np.int64)
    for b in range(NB):
        for st in (0, 1):
            mx = max(len(per[c][b][st][0]) for c in range(C))
            cnt[b, st] = (mx + P - 1) // P

    n_chunks = [int(cnt[:, st].sum()) for st in (0, 1)]
    n_chunks_pad = [((n + GCH - 1) // GCH) * GCH if n else 0 for n in n_chunks]

    def wrap(idx_flat):
        g = len(idx_flat) // GROUP
        w = idx_flat.reshape(g, GROUP // 16, 16)
        w = np.transpose(w, (0, 2, 1))
        return np.tile(w, (1, 8, 1)).astype(np.int16)

    def dlocw(dl_flat):
        g = len(dl_flat) // GROUP
        d = dl_flat.reshape(g, GCH, P)
        return np.transpose(d, (0, 2, 1)).astype(np.float32)

    idx_w, dloc_w = [], []
    for c in range(C):
        sidx = [[], []]
        sdl = [[], []]
        for b in range(NB):
            for st in (0, 1):
                want = cnt[b, st] * P
                ii, dd = per[c][b][st]
                padn = want - len(ii)
                sidx[st].append(np.concatenate([ii, np.zeros(padn, np.int64)]))
                sdl[st].append(np.concatenate([dd, -np.ones(padn, np.int64)]))
        iw, dw = [], []
        for st in (0, 1):
            arr_i = np.concatenate(sidx[st]) if sidx[st] else np.zeros(0, np.int64)
            arr_d = np.concatenate(sdl[st]) if sdl[st] else np.zeros(0, np.int64)
            tail = n_chunks_pad[st] * P - len(arr_i)
            arr_i = np.concatenate([arr_i, np.zeros(tail, np.int64)])
            arr_d = np.concatenate([arr_d, -np.ones(tail, np.int64)])
            iw.append(wrap(arr_i))
            dw.append(dlocw(arr_d))
        idx_w.append(iw)
        dloc_w.append(dw)

    return dict(
        LOCAL=LOCAL, NB=NB, PADL=PADL, C=C,
        H0B=H0B, H1B=H1B, TOT0=TOT0, TOT1=TOT1,
        cnt=cnt, n_chunks_pad=n_chunks_pad,
        idx_w=idx_w, dloc_w=dloc_w, dinv=dinv, sdeg=sdeg,
    )


def _host_tensors(pp, x, weights):
    C, LOCAL, PADL, NB = pp["C"], pp["LOCAL"], pp["PADL"], pp["NB"]
    W1, b1, W2, b2, W3, b3, Wl, bl = weights
    iota = np.tile(np.arange(P, dtype=np.float32), (P, 1))
    ident = np.eye(P, dtype=np.float32)
    ones = np.ones((1, P), np.float32)
    maps = []
    for c in range(C):
        xs = np.zeros((PADL, P), np.float32)
        xs[:LOCAL] = x[c * LOCAL:(c + 1) * LOCAL]
        dvl = np.zeros(PADL, np.float32)
        dvl[:LOCAL] = pp["dinv"][c * LOCAL:(c + 1) * LOCAL]
        dv = np.ascontiguousarray(dvl.reshape(NB, P).T)
        rd = np.zeros((1, PADL), np.float32)
        rd[0, :LOCAL] = pp["sdeg"][c * LOCAL:(c + 1) * LOCAL]
        m = {
            "xt": np.ascontiguousarray(xs.T),
            "w1": np.ascontiguousarray(W1, np.float32),
            "w2": np.ascontiguousarray(W2, np.float32),
            "w3": np.ascontiguousarray(W3, np.float32),
            "wl": np.ascontiguousarray(Wl, np.float32),
            "b1": np.asarray(b1, np.float32).reshape(1, -1),
            "b2": np.asarray(b2, np.float32).reshape(1, -1),
            "b3": np.asarray(b3, np.float32).reshape(1, -1),
            "bl": np.asarray(bl, np.float32).reshape(1, -1),
            "dinv_sb": dv, "recipd": rd,
            "iota": iota, "ident": ident, "ones": ones,
        }
        for st in (0, 1):
            if pp["n_chunks_pad"][st]:
                m[f"idx{st}"] = pp["idx_w"][c][st]
                m[f"dloc{st}"] = pp["dloc_w"][c][st]
        maps.append(m)
    return maps


def _build(pp, DOUT_, n_cores):
    NB, PADL = pp["NB"], pp["PADL"]
    H0B, H1B, TOT0, TOT1 = pp["H0B"], pp["H1B"], pp["TOT0"], pp["TOT1"]
    cnt, n_chunks_pad = pp["cnt"], pp["n_chunks_pad"]
    f32 = mybir.dt.float32
    f16 = mybir.dt.float16

    nc = bacc.Bacc("TRN2", target_bir_lowering=False, debug=False,
                   num_devices=n_cores, num_swdge_queues=NQ)

    xt = nc.dram_tensor("xt", [P, PADL], f32, kind="ExternalInput")
    w = [nc.dram_tensor(f"w{i+1}", [P, P], f32, kind="ExternalInput") for i in range(3)]
    wl = nc.dram_tensor("wl", [P, DOUT_], f32, kind="ExternalInput")
    bias = [nc.dram_tensor(f"b{i+1}", [1, P], f32, kind="ExternalInput") for i in range(3)]
    bl = nc.dram_tensor("bl", [1, DOUT_], f32, kind="ExternalInput")
    dinv_sb_d = nc.dram_tensor("dinv_sb", [P, NB], f32, kind="ExternalInput")
    recipd_d = nc.dram_tensor("recipd", [1, PADL], f32, kind="ExternalInput")
    iota_d = nc.dram_tensor("iota", [P, P], f32, kind="ExternalInput")
    ident_d = nc.dram_tensor("ident", [P, P], f32, kind="ExternalInput")
    ones_d = nc.dram_tensor("ones", [1, P], f32, kind="ExternalInput")
    idx_d, dloc_d = [None, None], [None, None]
    for st in (0, 1):
        g = n_chunks_pad[st] // GCH
        if g:
            idx_d[st] = nc.dram_tensor(f"idx{st}", [g, P, GROUP // 16],
                                       mybir.dt.int16, kind="ExternalInput")
            dloc_d[st] = nc.dram_tensor(f"dloc{st}", [g, P, GCH], f32,
                                        kind="ExternalInput")
    out_d = nc.dram_tensor("out", [PADL, DOUT_], f32, kind="ExternalOutput")

    rg = [list(range(n_cores))]

    from contextlib import ExitStack
    with tile.TileContext(nc) as tc, ExitStack() as ctx:
        dram = ctx.enter_context(tc.tile_pool(name="dram", bufs=1, space="DRAM"))
        cpool = ctx.enter_context(tc.tile_pool(name="consts", bufs=1))
        hpool = ctx.enter_context(tc.tile_pool(name="ht", bufs=1))
        mpool = ctx.enter_context(tc.tile_pool(name="msg", bufs=22))
        spool = ctx.enter_context(tc.tile_pool(name="sel", bufs=4))
        dpool = ctx.enter_context(tc.tile_pool(name="dloc", bufs=3))
        ipool = ctx.enter_context(tc.tile_pool(name="idx", bufs=3))
        zpool = ctx.enter_context(tc.tile_pool(name="zt", bufs=3))
        opool = ctx.enter_context(tc.tile_pool(name="outs", bufs=3))
        pz = ctx.enter_context(tc.tile_pool(name="pz", bufs=2, space="PSUM"))
        pa = ctx.enter_context(tc.tile_pool(name="pa", bufs=3, space="PSUM"))
        pt = ctx.enter_context(tc.tile_pool(name="pt", bufs=2, space="PSUM"))

        nc.gpsimd.load_library(mlp_lib)

        def const(dram_t, shape):
            t = cpool.tile(shape, f32, name=dram_t.name + "_sb")
            nc.sync.dma_start(t[:], dram_t[:])
            return t
        w_sb = [const(w[i], [P, P]) for i in range(3)]
        wl_sb = const(wl, [P, DOUT_])
        b_sb = [const(bias[i], [1, P]) for i in range(3)]
        bl_sb = const(bl, [1, DOUT_])
        dinv_sb = const(dinv_sb_d, [P, NB])
        recipd = const(recipd_d, [1, PADL])
        iota = const(iota_d, [P, P])
        ident = const(ident_d, [P, P])
        ones = const(ones_d, [1, P])

        hT = [hpool.tile([P, PADL], f32, name=f"hT{i}") for i in range(2)]
        nc.sync.dma_start(hT[0][:], xt[:])

        slab = [[dram.tile([H0B * P, P], f16, name=f"slab0_{i}"),
                 dram.tile([H1B * P, P], f16, name=f"slab1_{i}")]
                for i in range(3)]
        table = [[dram.tile([TOT0, P], f16, addr_space="Shared",
                            name=f"table0_{i}"),
                  dram.tile([TOT1, P], f16, addr_space="Shared",
                            name=f"table1_{i}")]
                 for i in range(3)]

        gq = [0]

        def z_block(layer, hTsrc, b):
            """Emit Z~ matmul for block b of `layer`, write to layer's slab."""
            buf = layer
            zp = pz.tile([P, P], f32, tag="z")
            nc.tensor.matmul(zp[:], lhsT=hTsrc[:, b * P:(b + 1) * P],
                             rhs=w_sb[layer][:], start=True, stop=True)
            zt = zpool.tile([P, P], f16, tag="zt")
            nc.scalar.activation(zt[:], zp[:],
                                 mybir.ActivationFunctionType.Copy,
                                 scale=dinv_sb[:, b:b + 1])
            if b < H0B:
                nc.sync.dma_start(slab[buf][0][b * P:(b + 1) * P, :], zt[:])
            else:
                bb = b - H0B
                nc.sync.dma_start(slab[buf][1][bb * P:(bb + 1) * P, :], zt[:])

        def ag(layer, half):
            buf = layer
            nc.gpsimd.collective_compute(
                "AllGather", mybir.AluOpType.bypass, replica_groups=rg,
                ins=[slab[buf][half].opt()], outs=[table[buf][half].opt()],
            )

        def agg_phase(layer, hTcur, hTnext, final=False):
            """Aggregation for `layer`; also emits layer+1's Z~/AG per block."""
            bm = bl_sb if final else b_sb[layer]
            ndout = DOUT_ if final else P
            buf = layer
            batch_tiles = [{}, {}]

            def ensure_batch(st, bt):
                if bt in batch_tiles[st]:
                    return batch_tiles[st][bt]
                g0 = bt * SB
                ng = min(SB, n_chunks_pad[st] // GCH - g0)
                nchk = ng * GCH
                it = ipool.tile([P, ng * (GROUP // 16)], mybir.dt.int16, tag="idx")
                nc.sync.dma_start(
                    it[:].rearrange("p (g m) -> p g m", m=GROUP // 16),
                    idx_d[st][g0:g0 + ng].rearrange("g p m -> p g m"),
                )
                dlt = dpool.tile([P, nchk], f32, tag="dloc")
                nc.sync.dma_start(
                    dlt[:].rearrange("p (g m) -> p g m", m=GCH),
                    dloc_d[st][g0:g0 + ng].rearrange("g p m -> p g m"),
                )
                st_t = spool.tile([P, nchk * P], f16, tag="sel")
                nc.vector.tensor_tensor(
                    out=st_t[:].rearrange("p (m j) -> p m j", j=P),
                    in0=dlt[:].to_broadcast([P, nchk, P]),
                    in1=iota[:].unsqueeze(1).to_broadcast([P, nchk, P]),
                    op=mybir.AluOpType.is_equal,
                )
                batch_tiles[st][bt] = (it, st_t, g0)
                return batch_tiles[st][bt]

            msg_tiles = [{}, {}]

            def ensure_group(st, g):
                if g in msg_tiles[st]:
                    return msg_tiles[st][g]
                it, _, g0 = ensure_batch(st, g // SB)
                mt = mpool.tile([P, GROUP], f16, tag="msg")
                iw = GROUP // 16
                nc.gpsimd.dma_gather(
                    out_ap=mt[:].rearrange("p (c e) -> p c e", e=P),
                    in_ap=table[buf][st][:],
                    idxs_ap=it[:, (g - g0) * iw:(g - g0 + 1) * iw],
                    num_idxs=GROUP,
                    num_idxs_reg=GROUP,
                    elem_size=P,
                    queue_num=gq[0] % NQ,
                )
                gq[0] += 1
                msg_tiles[st][g] = mt
                return mt

            cursor = [0, 0]
            for b in range(NB):
                ap = pa.tile([P, ndout], f32, tag="agg")
                first = True
                if not final:
                    for st in (0, 1):
                        for _ in range(int(cnt[b, st])):
                            ci = cursor[st]
                            cursor[st] += 1
                            g, col = ci // GCH, ci % GCH
                            mt = ensure_group(st, g)
                            _, sl, g0 = ensure_batch(st, g // SB)
                            scol = (g - g0) * GCH + col
                            nc.tensor.matmul(
                                ap[:],
                                lhsT=sl[:, scol * P:(scol + 1) * P],
                                rhs=mt[:, col * P:(col + 1) * P],
                                start=first, stop=False,
                            )
                            first = False
                lhs_b = ones[:] if final else recipd[0:1, b * P:(b + 1) * P]
                if final:
                    nc.tensor.matmul(ap[:], lhsT=hTcur[:, b * P:(b + 1) * P],
                                     rhs=wl_sb[:], start=first, stop=False)
                    first = False
                nc.tensor.matmul(ap[:], lhsT=lhs_b, rhs=bm[:],
                                 start=first, stop=True)
                if final:
                    ot = opool.tile([P, DOUT_], f32, tag="o")
                    nc.scalar.activation(ot[:], ap[:],
                                         mybir.ActivationFunctionType.Copy)
                    nc.sync.dma_start(out_d[b * P:(b + 1) * P, :], ot[:])
                else:
                    hn = zpool.tile([P, P], f32, tag="hn")
                    nc.scalar.activation(hn[:], ap[:],
                                         mybir.ActivationFunctionType.Relu,
                                         scale=dinv_sb[:, b:b + 1])
                    tp = pt.tile([P, P], f32, tag="tp")
                    nc.tensor.transpose(out=tp[:], in_=hn[:], identity=ident[:])
                    nc.scalar.activation(hTnext[:, b * P:(b + 1) * P], tp[:],
                                         mybir.ActivationFunctionType.Copy)
                    if layer < 2:
                        # next layer's Z~ for this block; fire AGs when a
                        # half-slab completes
                        z_block(layer + 1, hTnext, b)
                        if b == H0B - 1:
                            ag(layer + 1, 0)
                        elif b == NB - 1:
                            ag(layer + 1, 1)

        # prologue: layer 0 Z~ from x
        for b in range(NB):
            z_block(0, hT[0], b)
            if b == H0B - 1:
                ag(0, 0)
            elif b == NB - 1:
                ag(0, 1)

        for layer in range(3):
            agg_phase(layer, hT[layer % 2], hT[(layer + 1) % 2])
        agg_phase(3, hT[1], None, final=True)

    nc.compile()
    return nc


_CACHE = {}


def _get_compiled(edge_index):
    key = hash(np.asarray(edge_index, np.int64).tobytes())
    if key not in _CACHE:
        pp = _preprocess(edge_index, N_NODES, N_CORES)
        nc = _build(pp, DOUT, N_CORES)
        _CACHE[key] = (pp, nc)
    return _CACHE[key]


_LAST_RUN = {}


def kernel(x, edge_index, W1, b1, W2, b2, W3, b3, Wl, bl):
    x = np.asarray(x, np.float32)
    pp, nc = _get_compiled(edge_index)
    maps = _host_tensors(pp, x, (W1, b1, W2, b2, W3, b3, Wl, bl))

    from concourse.bass_utils import run_bass_kernel_spmd
    res = run_bass_kernel_spmd(nc, maps, core_ids=list(range(N_CORES)))
    LOCAL = pp["LOCAL"]
    out = np.concatenate(
        [np.asarray(res.results[c]["out"])[:LOCAL] for c in range(N_CORES)])
    _LAST_RUN["nc"] = nc
    _LAST_RUN["maps"] = maps
    return out


def _install_ntff_hook():
    """The agent image's antenv lacks axon_hooks; recreate it from the boot
    helper so run_bass_kernel_spmd(trace=True) can capture NTFF profiles."""
    import types
    if "antenv.axon_hooks" in sys.modules:
        return
    mod = types.ModuleType("antenv.axon_hooks")
    _state = {}
    mod.set_axon_ntff_profile_hook = lambda h: _state.__setitem__("h", h)
    mod.get_axon_ntff_profile_hook = lambda: _state.get("h")
    sys.modules["antenv.axon_hooks"] = mod
    import antenv
    antenv.axon_hooks = mod
    from trn_agent_boot.trn_boot import _ntff_profile_via_ctypes
    mod.set_axon_ntff_profile_hook(
        _ntff_profile_via_ctypes("/opt/axon/libaxon_pjrt.so"))


def profile_exec_ns():
    """Re-run the last kernel invocation with NTFF tracing; return exec ns."""
    if "nc" not in _LAST_RUN:
        return None
    _install_ntff_hook()
    from concourse.bass_utils import run_bass_kernel_spmd
    res = run_bass_kernel_spmd(
        _LAST_RUN["nc"], _LAST_RUN["maps"],
        core_ids=list(range(N_CORES)), trace=True,
    )
    _LAST_RUN["trace_res"] = res
    return res.exec_time_ns


# revision 8
# speedup vs baseline: 2.0305x; 1.0442x over previous
"""3-layer GCN + linear head on 8 TRN2 NeuronCores (Bass/Tile, SPMD).

Self-contained: hardcodes N=50000, E=600000, D=128, DOUT=32, 8 cores.

Math (matches the reference):
    src,dst + self-loops; deg = in-degree; dinv = rsqrt(deg)
    norm_e = dinv[src]*dinv[dst]
    layer(h): agg[d] = sum_e norm_e (hW)[src_e]; relu(agg+b)
    out = h3 @ Wl + bl

Device mapping: nodes sharded into 8 contiguous slabs (graph parallel).
Per layer: local Z~ = dinv ⊙ (H @ W) matmul -> AllGather the node table ->
bulk dma_gather of source rows (edges sorted by destination) -> segment-sum
via one-hot selection matmuls (sel built on DVE by iota compare)
accumulating per-128-destination-block PSUM -> relu + dinv scale (bias
folded in as a rank-1 matmul) -> PE transpose feeds the next layer's lhsT.

The node table is split in two halves (blocks 0..23 / 24..48 of each slab)
with separate AllGathers; each half-table has < 32768 rows so int16 gather
indices address it directly, and the second AllGather overlaps with the
next layer's first-half gathers. The next layer's Z~ matmul for block b is
emitted right after block b's aggregation closes, so each AllGather starts
as soon as its half-slab is ready — collectives run concurrently with the
tail of the previous aggregation phase.
"""
import sys
sys.path.insert(0, '/opt/trn_rl_repo')
import numpy as np

import concourse.bass as bass
import concourse.tile as tile
import concourse.mybir as mybir
from concourse import bacc
from concourse.library_config import mlp as mlp_lib

P = 128
GROUP = 1024      # edges per dma_gather (SWDGE ring carveout = 1024 desc)
GCH = GROUP // P
SB = 4            # gather groups per sel/idx batch
NQ = 4            # SWDGE queues

N_NODES = 50000
N_CORES = 8
DIN = 128
DOUT = 32


def _preprocess(edge_index, N, C):
    LOCAL = N // C
    NB = (LOCAL + P - 1) // P
    PADL = NB * P
    H0B = NB // 2
    H1B = NB - H0B
    H0R, H1R = H0B * P, H1B * P          # per-core rows per half
    TOT0, TOT1 = C * H0R, C * H1R        # table rows per half
    assert TOT0 <= 32768 and TOT1 <= 32768

    src = np.asarray(edge_index[0], dtype=np.int64)
    dst = np.asarray(edge_index[1], dtype=np.int64)
    loops = np.arange(N, dtype=np.int64)
    src = np.concatenate([src, loops])
    dst = np.concatenate([dst, loops])

    deg = np.bincount(dst, minlength=N).astype(np.float64)
    dinv = (1.0 / np.sqrt(deg)).astype(np.float32)
    sdeg = np.sqrt(deg).astype(np.float32)

    # source stream (which half-table) + id within that half-table
    score = src // LOCAL
    slocal = src % LOCAL
    s_st = (slocal >= H0R).astype(np.int64)

    core = dst // LOCAL
    ldst = dst % LOCAL

    # ---- balance blocks: permute nodes within each half so per-(block,
    # stream) in-edge counts are as even as possible (cuts chunk padding,
    # which is shared across cores). Node halves stay fixed, so the
    # src-stream labels (s_st) remain valid.
    perm = np.zeros((C, PADL), np.int64)      # slot -> orig local id (-1 pad)
    slot_of = np.zeros((C, LOCAL), np.int64)  # orig local id -> slot
    for c in range(C):
        m = core == c
        a_cnt = np.bincount(ldst[m & (s_st == 0)], minlength=LOCAL)
        b_cnt = np.bincount(ldst[m & (s_st == 1)], minlength=LOCAL)
        tot = a_cnt + b_cnt
        for h, (lo, hi, b0, nb) in enumerate(
                [(0, H0R, 0, H0B), (H0R, LOCAL, H0B, H1B)]):
            nodes = np.arange(lo, hi)
            order = np.argsort(-tot[nodes], kind="stable")
            nodes = nodes[order]
            cap = np.full(nb, P, np.int64)
            if h == 1:
                cap[-1] = LOCAL - H0R - (nb - 1) * P   # pad slots in last blk
            load = np.zeros(nb, np.float64)
            fill = np.zeros(nb, np.int64)
            for n in nodes:
                cand = np.where(fill < cap)[0]
                j = cand[np.argmin(load[cand])]
                blk = b0 + j
                slot = blk * P + fill[j]
                fill[j] += 1
                load[j] += tot[n]
                slot_of[c, n] = slot
                perm[c, slot] = n
            base = b0 * P
            for j in range(nb):
                for k in range(fill[j], P):
                    perm[c, base + j * P + k] = -1

    # recompute table ids from permuted slots
    sslot = slot_of[score, slocal]
    sid = np.where(s_st == 0, score * H0R + sslot,
                   score * H1R + (sslot - H0R))

    per = [[[None, None] for _ in range(NB)] for _ in range(C)]
    for c in range(C):
        m = core == c
        cl = slot_of[c][ldst[m]]
        cs, cst = sid[m], s_st[m]
        order = np.argsort(cl, kind="stable")
        cs, cl, cst = cs[order], cl[order], cst[order]
        blk = cl // P
        dl = cl % P
        for b in range(NB):
            bm = blk == b
            for st in (0, 1):
                sm = bm & (cst == st)
                per[c][b][st] = (cs[sm], dl[sm])

    cnt = np.zeros((NB, 2), dtype=np.int64)
    for b in range(NB):
        for st in (0, 1):
            mx = max(len(per[c][b][st][0]) for c in range(C))
            cnt[b, st] = (mx + P - 1) // P

    n_chunks = [int(cnt[:, st].sum()) for st in (0, 1)]
    n_chunks_pad = [((n + GCH - 1) // GCH) * GCH if n else 0 for n in n_chunks]

    def wrap(idx_flat):
        g = len(idx_flat) // GROUP
        w = idx_flat.reshape(g, GROUP // 16, 16)
        w = np.transpose(w, (0, 2, 1))
        return np.tile(w, (1, 8, 1)).astype(np.int16)

    def dlocw(dl_flat):
        g = len(dl_flat) // GROUP
        d = dl_flat.reshape(g, GCH, P)
        return np.transpose(d, (0, 2, 1)).astype(np.float32)

    idx_w, dloc_w = [], []
    for c in range(C):
        sidx = [[], []]
        sdl = [[], []]
        for b in range(NB):
            for st in (0, 1):
                want = cnt[b, st] * P
                ii, dd = per[c][b][st]
                padn = want - len(ii)
                sidx[st].append(np.concatenate([ii, np.zeros(padn, np.int64)]))
                sdl[st].append(np.concatenate([dd, -np.ones(padn, np.int64)]))
        iw, dw = [], []
        for st in (0, 1):
            arr_i = np.concatenate(sidx[st]) if sidx[st] else np.zeros(0, np.int64)
            arr_d = np.concatenate(sdl[st]) if sdl[st] else np.zeros(0, np.int64)
            tail = n_chunks_pad[st] * P - len(arr_i)
            arr_i = np.concatenate([arr_i, np.zeros(tail, np.int64)])
            arr_d = np.concatenate([arr_d, -np.ones(tail, np.int64)])
            iw.append(wrap(arr_i))
            dw.append(dlocw(arr_d))
        idx_w.append(iw)
        dloc_w.append(dw)

    return dict(
        LOCAL=LOCAL, NB=NB, PADL=PADL, C=C,
        H0B=H0B, H1B=H1B, TOT0=TOT0, TOT1=TOT1,
        cnt=cnt, n_chunks_pad=n_chunks_pad,
        idx_w=idx_w, dloc_w=dloc_w, dinv=dinv, sdeg=sdeg, perm=perm,
    )


def _host_tensors(pp, x, weights):
    C, LOCAL, PADL, NB = pp["C"], pp["LOCAL"], pp["PADL"], pp["NB"]
    W1, b1, W2, b2, W3, b3, Wl, bl = weights
    iota = np.tile(np.arange(P, dtype=np.float32), (P, 1))
    ident = np.eye(P, dtype=np.float32)
    ones = np.ones((1, P), np.float32)
    maps = []
    for c in range(C):
        pm = pp["perm"][c]
        valid = pm >= 0
        pmv = np.where(valid, pm, 0)
        xs = np.where(valid[:, None], x[c * LOCAL + pmv], 0).astype(np.float32)
        dvl = np.where(valid, pp["dinv"][c * LOCAL + pmv], 0).astype(np.float32)
        dv = np.ascontiguousarray(dvl.reshape(NB, P).T)
        rd = np.where(valid, pp["sdeg"][c * LOCAL + pmv], 0
                      ).astype(np.float32).reshape(1, -1)
        m = {
            "xt": np.ascontiguousarray(xs.T),
            "w1": np.ascontiguousarray(W1, np.float32),
            "w2": np.ascontiguousarray(W2, np.float32),
            "w3": np.ascontiguousarray(W3, np.float32),
            "wl": np.ascontiguousarray(Wl, np.float32),
            "b1": np.asarray(b1, np.float32).reshape(1, -1),
            "b2": np.asarray(b2, np.float32).reshape(1, -1),
            "b3": np.asarray(b3, np.float32).reshape(1, -1),
            "bl": np.asarray(bl, np.float32).reshape(1, -1),
            "dinv_sb": dv, "recipd": rd,
            "iota": iota, "ident": ident, "ones": ones,
        }
        for st in (0, 1):
            if pp["n_chunks_pad"][st]:
                m[f"idx{st}"] = pp["idx_w"][c][st]
                m[f"dloc{st}"] = pp["dloc_w"][c][st]
        maps.append(m)
    return maps


def _build(pp, DOUT_, n_cores):
    NB, PADL = pp["NB"], pp["PADL"]
    H0B, H1B, TOT0, TOT1 = pp["H0B"], pp["H1B"], pp["TOT0"], pp["TOT1"]
    cnt, n_chunks_pad = pp["cnt"], pp["n_chunks_pad"]
    f32 = mybir.dt.float32
    f16 = mybir.dt.float16

    nc = bacc.Bacc("TRN2", target_bir_lowering=False, debug=False,
                   num_devices=n_cores, num_swdge_queues=NQ)

    xt = nc.dram_tensor("xt", [P, PADL], f32, kind="ExternalInput")
    w = [nc.dram_tensor(f"w{i+1}", [P, P], f32, kind="ExternalInput") for i in range(3)]
    wl = nc.dram_tensor("wl", [P, DOUT_], f32, kind="ExternalInput")
    bias = [nc.dram_tensor(f"b{i+1}", [1, P], f32, kind="ExternalInput") for i in range(3)]
    bl = nc.dram_tensor("bl", [1, DOUT_], f32, kind="ExternalInput")
    dinv_sb_d = nc.dram_tensor("dinv_sb", [P, NB], f32, kind="ExternalInput")
    recipd_d = nc.dram_tensor("recipd", [1, PADL], f32, kind="ExternalInput")
    iota_d = nc.dram_tensor("iota", [P, P], f32, kind="ExternalInput")
    ident_d = nc.dram_tensor("ident", [P, P], f32, kind="ExternalInput")
    ones_d = nc.dram_tensor("ones", [1, P], f32, kind="ExternalInput")
    idx_d, dloc_d = [None, None], [None, None]
    for st in (0, 1):
        g = n_chunks_pad[st] // GCH
        if g:
            idx_d[st] = nc.dram_tensor(f"idx{st}", [g, P, GROUP // 16],
                                       mybir.dt.int16, kind="ExternalInput")
            dloc_d[st] = nc.dram_tensor(f"dloc{st}", [g, P, GCH], f32,
                                        kind="ExternalInput")
    out_d = nc.dram_tensor("out", [PADL, DOUT_], f32, kind="ExternalOutput")

    rg = [list(range(n_cores))]

    from contextlib import ExitStack
    with tile.TileContext(nc) as tc, ExitStack() as ctx:
        dram = ctx.enter_context(tc.tile_pool(name="dram", bufs=1, space="DRAM"))
        cpool = ctx.enter_context(tc.tile_pool(name="consts", bufs=1))
        hpool = ctx.enter_context(tc.tile_pool(name="ht", bufs=1))
        mpool = ctx.enter_context(tc.tile_pool(name="msg", bufs=22))
        spool = ctx.enter_context(tc.tile_pool(name="sel", bufs=4))
        dpool = ctx.enter_context(tc.tile_pool(name="dloc", bufs=3))
        ipool = ctx.enter_context(tc.tile_pool(name="idx", bufs=3))
        zpool = ctx.enter_context(tc.tile_pool(name="zt", bufs=3))
        opool = ctx.enter_context(tc.tile_pool(name="outs", bufs=3))
        pz = ctx.enter_context(tc.tile_pool(name="pz", bufs=2, space="PSUM"))
        pa = ctx.enter_context(tc.tile_pool(name="pa", bufs=3, space="PSUM"))
        pt = ctx.enter_context(tc.tile_pool(name="pt", bufs=2, space="PSUM"))

        nc.gpsimd.load_library(mlp_lib)

        def const(dram_t, shape):
            t = cpool.tile(shape, f32, name=dram_t.name + "_sb")
            nc.sync.dma_start(t[:], dram_t[:])
            return t
        w_sb = [const(w[i], [P, P]) for i in range(3)]
        wl_sb = const(wl, [P, DOUT_])
        b_sb = [const(bias[i], [1, P]) for i in range(3)]
        bl_sb = const(bl, [1, DOUT_])
        dinv_sb = const(dinv_sb_d, [P, NB])
        recipd = const(recipd_d, [1, PADL])
        iota = const(iota_d, [P, P])
        ident = const(ident_d, [P, P])
        ones = const(ones_d, [1, P])

        hT = [hpool.tile([P, PADL], f32, name=f"hT{i}") for i in range(2)]
        nc.sync.dma_start(hT[0][:], xt[:])

        slab = [[dram.tile([H0B * P, P], f16, name=f"slab0_{i}"),
                 dram.tile([H1B * P, P], f16, name=f"slab1_{i}")]
                for i in range(3)]
        table = [[dram.tile([TOT0, P], f16, addr_space="Shared",
                            name=f"table0_{i}"),
                  dram.tile([TOT1, P], f16, addr_space="Shared",
                            name=f"table1_{i}")]
                 for i in range(3)]

        gq = [0]

        def z_block(layer, hTsrc, b):
            """Emit Z~ matmul for block b of `layer`, write to layer's slab."""
            buf = layer
            zp = pz.tile([P, P], f32, tag="z")
            nc.tensor.matmul(zp[:], lhsT=hTsrc[:, b * P:(b + 1) * P],
                             rhs=w_sb[layer][:], start=True, stop=True)
            zt = zpool.tile([P, P], f16, tag="zt")
            nc.scalar.activation(zt[:], zp[:],
                                 mybir.ActivationFunctionType.Copy,
                                 scale=dinv_sb[:, b:b + 1])
            if b < H0B:
                nc.sync.dma_start(slab[buf][0][b * P:(b + 1) * P, :], zt[:])
            else:
                bb = b - H0B
                nc.sync.dma_start(slab[buf][1][bb * P:(bb + 1) * P, :], zt[:])

        def ag(layer, half):
            buf = layer
            nc.gpsimd.collective_compute(
                "AllGather", mybir.AluOpType.bypass, replica_groups=rg,
                ins=[slab[buf][half].opt()], outs=[table[buf][half].opt()],
            )

        def agg_phase(layer, hTcur, hTnext, final=False):
            """Aggregation for `layer`; also emits layer+1's Z~/AG per block."""
            bm = bl_sb if final else b_sb[layer]
            ndout = DOUT_ if final else P
            buf = layer
            batch_tiles = [{}, {}]

            def ensure_batch(st, bt):
                if bt in batch_tiles[st]:
                    return batch_tiles[st][bt]
                g0 = bt * SB
                ng = min(SB, n_chunks_pad[st] // GCH - g0)
                nchk = ng * GCH
                it = ipool.tile([P, ng * (GROUP // 16)], mybir.dt.int16, tag="idx")
                nc.sync.dma_start(
                    it[:].rearrange("p (g m) -> p g m", m=GROUP // 16),
                    idx_d[st][g0:g0 + ng].rearrange("g p m -> p g m"),
                )
                dlt = dpool.tile([P, nchk], f32, tag="dloc")
                nc.sync.dma_start(
                    dlt[:].rearrange("p (g m) -> p g m", m=GCH),
                    dloc_d[st][g0:g0 + ng].rearrange("g p m -> p g m"),
                )
                st_t = spool.tile([P, nchk * P], f16, tag="sel")
                nc.vector.tensor_tensor(
                    out=st_t[:].rearrange("p (m j) -> p m j", j=P),
                    in0=dlt[:].to_broadcast([P, nchk, P]),
                    in1=iota[:].unsqueeze(1).to_broadcast([P, nchk, P]),
                    op=mybir.AluOpType.is_equal,
                )
                batch_tiles[st][bt] = (it, st_t, g0)
                return batch_tiles[st][bt]

            msg_tiles = [{}, {}]

            def ensure_group(st, g):
                if g in msg_tiles[st]:
                    return msg_tiles[st][g]
                it, _, g0 = ensure_batch(st, g // SB)
                mt = mpool.tile([P, GROUP], f16, tag="msg")
                iw = GROUP // 16
                nc.gpsimd.dma_gather(
                    out_ap=mt[:].rearrange("p (c e) -> p c e", e=P),
                    in_ap=table[buf][st][:],
                    idxs_ap=it[:, (g - g0) * iw:(g - g0 + 1) * iw],
                    num_idxs=GROUP,
                    num_idxs_reg=GROUP,
                    elem_size=P,
                    queue_num=gq[0] % NQ,
                )
                gq[0] += 1
                msg_tiles[st][g] = mt
                return mt

            cursor = [0, 0]
            for b in range(NB):
                ap = pa.tile([P, ndout], f32, tag="agg")
                first = True
                if not final:
                    for st in (0, 1):
                        for _ in range(int(cnt[b, st])):
                            ci = cursor[st]
                            cursor[st] += 1
                            g, col = ci // GCH, ci % GCH
                            mt = ensure_group(st, g)
                            _, sl, g0 = ensure_batch(st, g // SB)
                            scol = (g - g0) * GCH + col
                            nc.tensor.matmul(
                                ap[:],
                                lhsT=sl[:, scol * P:(scol + 1) * P],
                                rhs=mt[:, col * P:(col + 1) * P],
                                start=first, stop=False,
                            )
                            first = False
                lhs_b = ones[:] if final else recipd[0:1, b * P:(b + 1) * P]
                if final:
                    nc.tensor.matmul(ap[:], lhsT=hTcur[:, b * P:(b + 1) * P],
                                     rhs=wl_sb[:], start=first, stop=False)
                    first = False
                nc.tensor.matmul(ap[:], lhsT=lhs_b, rhs=bm[:],
                                 start=first, stop=True)
                if final:
                    ot = opool.tile([P, DOUT_], f32, tag="o")
                    nc.scalar.activation(ot[:], ap[:],
                                         mybir.ActivationFunctionType.Copy)
                    nc.sync.dma_start(out_d[b * P:(b + 1) * P, :], ot[:])
                else:
                    hn = zpool.tile([P, P], f32, tag="hn")
                    nc.scalar.activation(hn[:], ap[:],
                                         mybir.ActivationFunctionType.Relu,
                                         scale=dinv_sb[:, b:b + 1])
                    tp = pt.tile([P, P], f32, tag="tp")
                    nc.tensor.transpose(out=tp[:], in_=hn[:], identity=ident[:])
                    nc.scalar.activation(hTnext[:, b * P:(b + 1) * P], tp[:],
                                         mybir.ActivationFunctionType.Copy)
                    if layer < 2:
                        # next layer's Z~ for this block; fire AGs when a
                        # half-slab completes
                        z_block(layer + 1, hTnext, b)
                        if b == H0B - 1:
                            ag(layer + 1, 0)
                        elif b == NB - 1:
                            ag(layer + 1, 1)

        # prologue: layer 0 Z~ from x
        for b in range(NB):
            z_block(0, hT[0], b)
            if b == H0B - 1:
                ag(0, 0)
            elif b == NB - 1:
                ag(0, 1)

        for layer in range(3):
            agg_phase(layer, hT[layer % 2], hT[(layer + 1) % 2])
        agg_phase(3, hT[1], None, final=True)

    nc.compile()
    return nc


_CACHE = {}


def _get_compiled(edge_index):
    key = hash(np.asarray(edge_index, np.int64).tobytes())
    if key not in _CACHE:
        pp = _preprocess(edge_index, N_NODES, N_CORES)
        nc = _build(pp, DOUT, N_CORES)
        _CACHE[key] = (pp, nc)
    return _CACHE[key]


_LAST_RUN = {}


def kernel(x, edge_index, W1, b1, W2, b2, W3, b3, Wl, bl):
    x = np.asarray(x, np.float32)
    pp, nc = _get_compiled(edge_index)
    maps = _host_tensors(pp, x, (W1, b1, W2, b2, W3, b3, Wl, bl))

    from concourse.bass_utils import run_bass_kernel_spmd
    res = run_bass_kernel_spmd(nc, maps, core_ids=list(range(N_CORES)))
    LOCAL = pp["LOCAL"]
    parts = []
    for c in range(N_CORES):
        r = np.asarray(res.results[c]["out"])
        pm = pp["perm"][c]
        valid = pm >= 0
        o = np.zeros((LOCAL, r.shape[1]), r.dtype)
        o[pm[valid]] = r[valid]
        parts.append(o)
    out = np.concatenate(parts)
    _LAST_RUN["nc"] = nc
    _LAST_RUN["maps"] = maps
    return out


def _install_ntff_hook():
    """The agent image's antenv lacks axon_hooks; recreate it from the boot
    helper so run_bass_kernel_spmd(trace=True) can capture NTFF profiles."""
    import types
    if "antenv.axon_hooks" in sys.modules:
        return
    mod = types.ModuleType("antenv.axon_hooks")
    _state = {}
    mod.set_axon_ntff_profile_hook = lambda h: _state.__setitem__("h", h)
    mod.get_axon_ntff_profile_hook = lambda: _state.get("h")
    sys.modules["antenv.axon_hooks"] = mod
    import antenv
    antenv.axon_hooks = mod
    from trn_agent_boot.trn_boot import _ntff_profile_via_ctypes
    mod.set_axon_ntff_profile_hook(
        _ntff_profile_via_ctypes("/opt/axon/libaxon_pjrt.so"))


def profile_exec_ns():
    """Re-run the last kernel invocation with NTFF tracing; return exec ns."""
    if "nc" not in _LAST_RUN:
        return None
    _install_ntff_hook()
    from concourse.bass_utils import run_bass_kernel_spmd
    res = run_bass_kernel_spmd(
        _LAST_RUN["nc"], _LAST_RUN["maps"],
        core_ids=list(range(N_CORES)), trace=True,
    )
    _LAST_RUN["trace_res"] = res
    return res.exec_time_ns


# revision 12
# speedup vs baseline: 2.1673x; 1.0674x over previous
"""3-layer GCN + linear head on 8 TRN2 NeuronCores (Bass/Tile, SPMD).

Self-contained: hardcodes N=50000, E=600000, D=128, DOUT=32, 8 cores.

Math (matches the reference):
    src,dst + self-loops; deg = in-degree; dinv = rsqrt(deg)
    norm_e = dinv[src]*dinv[dst]
    layer(h): agg[d] = sum_e norm_e (hW)[src_e]; relu(agg+b)
    out = h3 @ Wl + bl

Device mapping: nodes sharded into 8 contiguous slabs (graph parallel).
Per layer: local Z~ = dinv ⊙ (H @ W) matmul -> AllGather the node table ->
bulk dma_gather of source rows (edges sorted by destination) -> segment-sum
via one-hot selection matmuls (sel built on DVE by iota compare)
accumulating per-128-destination-block PSUM -> relu + dinv scale (bias
folded in as a rank-1 matmul) -> PE transpose feeds the next layer's lhsT.

The node table is split in two halves (blocks 0..23 / 24..48 of each slab)
with separate AllGathers; each half-table has < 32768 rows so int16 gather
indices address it directly, and the second AllGather overlaps with the
next layer's first-half gathers. The next layer's Z~ matmul for block b is
emitted right after block b's aggregation closes, so each AllGather starts
as soon as its half-slab is ready — collectives run concurrently with the
tail of the previous aggregation phase.
"""
import sys
sys.path.insert(0, '/opt/trn_rl_repo')
import numpy as np

import concourse.bass as bass
import concourse.tile as tile
import concourse.mybir as mybir
from concourse import bacc
from concourse.library_config import mlp as mlp_lib

P = 128
GROUP = 1024      # edges per dma_gather (SWDGE ring carveout = 1024 desc)
GCH = GROUP // P
SB = 4            # gather groups per sel/idx batch
NQ = 4            # SWDGE queues

N_NODES = 50000
N_CORES = 8
DIN = 128
DOUT = 32


def _preprocess(edge_index, N, C):
    LOCAL = N // C
    NB = (LOCAL + P - 1) // P
    PADL = NB * P
    H0B = NB // 2
    H1B = NB - H0B
    H0R, H1R = H0B * P, H1B * P          # per-core rows per half
    TOT0, TOT1 = C * H0R, C * H1R        # table rows per half
    assert TOT0 <= 32768 and TOT1 <= 32768

    src = np.asarray(edge_index[0], dtype=np.int64)
    dst = np.asarray(edge_index[1], dtype=np.int64)
    loops = np.arange(N, dtype=np.int64)
    src = np.concatenate([src, loops])
    dst = np.concatenate([dst, loops])

    deg = np.bincount(dst, minlength=N).astype(np.float64)
    dinv = (1.0 / np.sqrt(deg)).astype(np.float32)
    sdeg = np.sqrt(deg).astype(np.float32)

    # source stream (which half-table) + id within that half-table
    score = src // LOCAL
    slocal = src % LOCAL
    s_st = (slocal >= H0R).astype(np.int64)

    core = dst // LOCAL
    ldst = dst % LOCAL

    # ---- balance blocks: permute nodes within each half so per-(block,
    # stream) in-edge counts are as even as possible (cuts chunk padding,
    # which is shared across cores). Node halves stay fixed, so the
    # src-stream labels (s_st) remain valid.
    perm = np.zeros((C, PADL), np.int64)      # slot -> orig local id (-1 pad)
    slot_of = np.zeros((C, LOCAL), np.int64)  # orig local id -> slot
    for c in range(C):
        m = core == c
        a_cnt = np.bincount(ldst[m & (s_st == 0)], minlength=LOCAL)
        b_cnt = np.bincount(ldst[m & (s_st == 1)], minlength=LOCAL)
        tot = a_cnt + b_cnt
        for h, (lo, hi, b0, nb) in enumerate(
                [(0, H0R, 0, H0B), (H0R, LOCAL, H0B, H1B)]):
            nodes = np.arange(lo, hi)
            order = np.argsort(-tot[nodes], kind="stable")
            nodes = nodes[order]
            cap = np.full(nb, P, np.int64)
            if h == 1:
                cap[-1] = LOCAL - H0R - (nb - 1) * P   # pad slots in last blk
            load = np.zeros(nb, np.float64)
            fill = np.zeros(nb, np.int64)
            for n in nodes:
                cand = np.where(fill < cap)[0]
                j = cand[np.argmin(load[cand])]
                blk = b0 + j
                slot = blk * P + fill[j]
                fill[j] += 1
                load[j] += tot[n]
                slot_of[c, n] = slot
                perm[c, slot] = n
            base = b0 * P
            for j in range(nb):
                for k in range(fill[j], P):
                    perm[c, base + j * P + k] = -1

    # recompute table ids from permuted slots
    sslot = slot_of[score, slocal]
    sid = np.where(s_st == 0, score * H0R + sslot,
                   score * H1R + (sslot - H0R))

    per = [[[None, None] for _ in range(NB)] for _ in range(C)]
    for c in range(C):
        m = core == c
        cl = slot_of[c][ldst[m]]
        cs, cst = sid[m], s_st[m]
        order = np.argsort(cl, kind="stable")
        cs, cl, cst = cs[order], cl[order], cst[order]
        blk = cl // P
        dl = cl % P
        for b in range(NB):
            bm = blk == b
            for st in (0, 1):
                sm = bm & (cst == st)
                per[c][b][st] = (cs[sm], dl[sm])

    cnt = np.zeros((NB, 2), dtype=np.int64)
    for b in range(NB):
        for st in (0, 1):
            mx = max(len(per[c][b][st][0]) for c in range(C))
            cnt[b, st] = (mx + P - 1) // P

    n_chunks = [int(cnt[:, st].sum()) for st in (0, 1)]
    n_chunks_pad = [((n + GCH - 1) // GCH) * GCH if n else 0 for n in n_chunks]

    def wrap(idx_flat):
        g = len(idx_flat) // GROUP
        w = idx_flat.reshape(g, GROUP // 16, 16)
        w = np.transpose(w, (0, 2, 1))
        return np.tile(w, (1, 8, 1)).astype(np.int16)

    def dlocw(dl_flat):
        g = len(dl_flat) // GROUP
        d = dl_flat.reshape(g, GCH, P)
        return np.transpose(d, (0, 2, 1)).astype(np.float32)

    idx_w, dloc_w = [], []
    for c in range(C):
        sidx = [[], []]
        sdl = [[], []]
        for b in range(NB):
            for st in (0, 1):
                want = cnt[b, st] * P
                ii, dd = per[c][b][st]
                padn = want - len(ii)
                sidx[st].append(np.concatenate([ii, np.zeros(padn, np.int64)]))
                sdl[st].append(np.concatenate([dd, -np.ones(padn, np.int64)]))
        iw, dw = [], []
        for st in (0, 1):
            arr_i = np.concatenate(sidx[st]) if sidx[st] else np.zeros(0, np.int64)
            arr_d = np.concatenate(sdl[st]) if sdl[st] else np.zeros(0, np.int64)
            tail = n_chunks_pad[st] * P - len(arr_i)
            arr_i = np.concatenate([arr_i, np.zeros(tail, np.int64)])
            arr_d = np.concatenate([arr_d, -np.ones(tail, np.int64)])
            iw.append(wrap(arr_i))
            dw.append(dlocw(arr_d))
        idx_w.append(iw)
        dloc_w.append(dw)

    return dict(
        LOCAL=LOCAL, NB=NB, PADL=PADL, C=C,
        H0B=H0B, H1B=H1B, TOT0=TOT0, TOT1=TOT1,
        cnt=cnt, n_chunks_pad=n_chunks_pad,
        idx_w=idx_w, dloc_w=dloc_w, dinv=dinv, sdeg=sdeg, perm=perm,
    )


def _host_tensors(pp, x, weights):
    C, LOCAL, PADL, NB = pp["C"], pp["LOCAL"], pp["PADL"], pp["NB"]
    W1, b1, W2, b2, W3, b3, Wl, bl = weights
    iota = np.tile(np.arange(P, dtype=np.float32), (P, 1))
    ident = np.eye(P, dtype=np.float32)
    ones = np.ones((1, P), np.float32)
    maps = []
    for c in range(C):
        pm = pp["perm"][c]
        valid = pm >= 0
        pmv = np.where(valid, pm, 0)
        xs = np.where(valid[:, None], x[c * LOCAL + pmv], 0).astype(np.float32)
        dvl = np.where(valid, pp["dinv"][c * LOCAL + pmv], 0).astype(np.float32)
        dv = np.ascontiguousarray(dvl.reshape(NB, P).T)
        rd = np.where(valid, pp["sdeg"][c * LOCAL + pmv], 0
                      ).astype(np.float32).reshape(1, -1)
        m = {
            "xt": np.ascontiguousarray(xs.T),
            "w1": np.ascontiguousarray(W1, np.float32),
            "w2": np.ascontiguousarray(W2, np.float32),
            "w3": np.ascontiguousarray(W3, np.float32),
            "wl": np.ascontiguousarray(Wl, np.float32),
            "b1": np.asarray(b1, np.float32).reshape(1, -1),
            "b2": np.asarray(b2, np.float32).reshape(1, -1),
            "b3": np.asarray(b3, np.float32).reshape(1, -1),
            "bl": np.asarray(bl, np.float32).reshape(1, -1),
            "dinv_sb": dv, "recipd": rd,
            "iota": iota, "ident": ident, "ones": ones,
        }
        for st in (0, 1):
            if pp["n_chunks_pad"][st]:
                m[f"idx{st}"] = pp["idx_w"][c][st]
                m[f"dloc{st}"] = pp["dloc_w"][c][st]
        maps.append(m)
    return maps


def _build(pp, DOUT_, n_cores):
    NB, PADL = pp["NB"], pp["PADL"]
    H0B, H1B, TOT0, TOT1 = pp["H0B"], pp["H1B"], pp["TOT0"], pp["TOT1"]
    cnt, n_chunks_pad = pp["cnt"], pp["n_chunks_pad"]
    f32 = mybir.dt.float32
    f16 = mybir.dt.float16

    nc = bacc.Bacc("TRN2", target_bir_lowering=False, debug=False,
                   num_devices=n_cores, num_swdge_queues=NQ)

    xt = nc.dram_tensor("xt", [P, PADL], f32, kind="ExternalInput")
    w = [nc.dram_tensor(f"w{i+1}", [P, P], f32, kind="ExternalInput") for i in range(3)]
    wl = nc.dram_tensor("wl", [P, DOUT_], f32, kind="ExternalInput")
    bias = [nc.dram_tensor(f"b{i+1}", [1, P], f32, kind="ExternalInput") for i in range(3)]
    bl = nc.dram_tensor("bl", [1, DOUT_], f32, kind="ExternalInput")
    dinv_sb_d = nc.dram_tensor("dinv_sb", [P, NB], f32, kind="ExternalInput")
    recipd_d = nc.dram_tensor("recipd", [1, PADL], f32, kind="ExternalInput")
    iota_d = nc.dram_tensor("iota", [P, P], f32, kind="ExternalInput")
    ident_d = nc.dram_tensor("ident", [P, P], f32, kind="ExternalInput")
    ones_d = nc.dram_tensor("ones", [1, P], f32, kind="ExternalInput")
    idx_d, dloc_d = [None, None], [None, None]
    for st in (0, 1):
        g = n_chunks_pad[st] // GCH
        if g:
            idx_d[st] = nc.dram_tensor(f"idx{st}", [g, P, GROUP // 16],
                                       mybir.dt.int16, kind="ExternalInput")
            dloc_d[st] = nc.dram_tensor(f"dloc{st}", [g, P, GCH], f32,
                                        kind="ExternalInput")
    out_d = nc.dram_tensor("out", [PADL, DOUT_], f32, kind="ExternalOutput")

    rg = [list(range(n_cores))]

    from contextlib import ExitStack
    with tile.TileContext(nc) as tc, ExitStack() as ctx:
        dram = ctx.enter_context(tc.tile_pool(name="dram", bufs=1, space="DRAM"))
        cpool = ctx.enter_context(tc.tile_pool(name="consts", bufs=1))
        hpool = ctx.enter_context(tc.tile_pool(name="ht", bufs=1))
        mpool = ctx.enter_context(tc.tile_pool(name="msg", bufs=22))
        spool = ctx.enter_context(tc.tile_pool(name="sel", bufs=4))
        dpool = ctx.enter_context(tc.tile_pool(name="dloc", bufs=3))
        ipool = ctx.enter_context(tc.tile_pool(name="idx", bufs=3))
        zpool = ctx.enter_context(tc.tile_pool(name="zt", bufs=3))
        opool = ctx.enter_context(tc.tile_pool(name="outs", bufs=3))
        pz = ctx.enter_context(tc.tile_pool(name="pz", bufs=2, space="PSUM"))
        pa = ctx.enter_context(tc.tile_pool(name="pa", bufs=3, space="PSUM"))
        pt = ctx.enter_context(tc.tile_pool(name="pt", bufs=2, space="PSUM"))

        nc.gpsimd.load_library(mlp_lib)

        def const(dram_t, shape):
            t = cpool.tile(shape, f32, name=dram_t.name + "_sb")
            nc.sync.dma_start(t[:], dram_t[:])
            return t
        w_sb = [const(w[i], [P, P]) for i in range(3)]
        wl_sb = const(wl, [P, DOUT_])
        b_sb = [const(bias[i], [1, P]) for i in range(3)]
        bl_sb = const(bl, [1, DOUT_])
        dinv_sb = const(dinv_sb_d, [P, NB])
        recipd = const(recipd_d, [1, PADL])
        iota = const(iota_d, [P, P])
        ident = const(ident_d, [P, P])
        ones = const(ones_d, [1, P])

        hT = [hpool.tile([P, PADL], f32, name=f"hT{i}") for i in range(2)]
        nc.sync.dma_start(hT[0][:], xt[:])

        slab = [[dram.tile([H0B * P, P], f16, name=f"slab0_{i}"),
                 dram.tile([H1B * P, P], f16, name=f"slab1_{i}")]
                for i in range(3)]
        table = [[dram.tile([TOT0, P], f16, addr_space="Shared",
                            name=f"table0_{i}"),
                  dram.tile([TOT1, P], f16, addr_space="Shared",
                            name=f"table1_{i}")]
                 for i in range(3)]

        gq = [0]

        ZB = 4            # z blocks batched per slab DMA
        zstate = {}

        def z_block(layer, hTsrc, b):
            """Emit Z~ matmul for block b of `layer`; slab DMA every ZB
            blocks (never straddling the half boundary)."""
            buf = layer
            zp = pz.tile([P, P], f32, tag="z")
            nc.tensor.matmul(zp[:], lhsT=hTsrc[:, b * P:(b + 1) * P],
                             rhs=w_sb[layer][:], start=True, stop=True)
            half = 0 if b < H0B else 1
            bb = b if half == 0 else b - H0B
            nblk = H0B if half == 0 else H1B
            j = bb % ZB
            if j == 0:
                zstate["t"] = zpool.tile([P, ZB * P], f16, tag="zt", name="ztb")
                zstate["b0"] = bb
            zt = zstate["t"]
            nc.scalar.activation(zt[:, j * P:(j + 1) * P], zp[:],
                                 mybir.ActivationFunctionType.Copy,
                                 scale=dinv_sb[:, b:b + 1])
            if j == ZB - 1 or bb == nblk - 1:
                nb_ = j + 1
                b0 = zstate["b0"]
                nc.sync.dma_start(
                    slab[buf][half][b0 * P:(b0 + nb_) * P, :].rearrange(
                        "(g p) f -> p g f", p=P),
                    zt[:, :nb_ * P].rearrange("p (g f) -> p g f", f=P),
                )

        def ag(layer, half):
            buf = layer
            nc.gpsimd.collective_compute(
                "AllGather", mybir.AluOpType.bypass, replica_groups=rg,
                ins=[slab[buf][half].opt()], outs=[table[buf][half].opt()],
            )

        def agg_phase(layer, hTcur, hTnext, final=False):
            """Aggregation for `layer`; also emits layer+1's Z~/AG per block."""
            bm = bl_sb if final else b_sb[layer]
            ndout = DOUT_ if final else P
            buf = layer
            batch_tiles = [{}, {}]

            def ensure_batch(st, bt):
                if bt in batch_tiles[st]:
                    return batch_tiles[st][bt]
                g0 = bt * SB
                ng = min(SB, n_chunks_pad[st] // GCH - g0)
                nchk = ng * GCH
                it = ipool.tile([P, ng * (GROUP // 16)], mybir.dt.int16, tag="idx")
                nc.sync.dma_start(
                    it[:].rearrange("p (g m) -> p g m", m=GROUP // 16),
                    idx_d[st][g0:g0 + ng].rearrange("g p m -> p g m"),
                )
                dlt = dpool.tile([P, nchk], f32, tag="dloc")
                nc.sync.dma_start(
                    dlt[:].rearrange("p (g m) -> p g m", m=GCH),
                    dloc_d[st][g0:g0 + ng].rearrange("g p m -> p g m"),
                )
                st_t = spool.tile([P, nchk * P], f16, tag="sel")
                nc.vector.tensor_tensor(
                    out=st_t[:].rearrange("p (m j) -> p m j", j=P),
                    in0=dlt[:].to_broadcast([P, nchk, P]),
                    in1=iota[:].unsqueeze(1).to_broadcast([P, nchk, P]),
                    op=mybir.AluOpType.is_equal,
                )
                batch_tiles[st][bt] = (it, st_t, g0)
                return batch_tiles[st][bt]

            msg_tiles = [{}, {}]

            def ensure_group(st, g):
                if g in msg_tiles[st]:
                    return msg_tiles[st][g]
                it, _, g0 = ensure_batch(st, g // SB)
                mt = mpool.tile([P, GROUP], f16, tag="msg")
                iw = GROUP // 16
                nc.gpsimd.dma_gather(
                    out_ap=mt[:].rearrange("p (c e) -> p c e", e=P),
                    in_ap=table[buf][st][:],
                    idxs_ap=it[:, (g - g0) * iw:(g - g0 + 1) * iw],
                    num_idxs=GROUP,
                    num_idxs_reg=GROUP,
                    elem_size=P,
                    queue_num=gq[0] % NQ,
                )
                gq[0] += 1
                msg_tiles[st][g] = mt
                return mt

            cursor = [0, 0]
            for b in range(NB):
                ap = pa.tile([P, ndout], f32, tag="agg")
                first = True
                if not final:
                    for st in (0, 1):
                        for _ in range(int(cnt[b, st])):
                            ci = cursor[st]
                            cursor[st] += 1
                            g, col = ci // GCH, ci % GCH
                            mt = ensure_group(st, g)
                            _, sl, g0 = ensure_batch(st, g // SB)
                            scol = (g - g0) * GCH + col
                            nc.tensor.matmul(
                                ap[:],
                                lhsT=sl[:, scol * P:(scol + 1) * P],
                                rhs=mt[:, col * P:(col + 1) * P],
                                start=first, stop=False,
                            )
                            first = False
                lhs_b = ones[:] if final else recipd[0:1, b * P:(b + 1) * P]
                if final:
                    nc.tensor.matmul(ap[:], lhsT=hTcur[:, b * P:(b + 1) * P],
                                     rhs=wl_sb[:], start=first, stop=False)
                    first = False
                nc.tensor.matmul(ap[:], lhsT=lhs_b, rhs=bm[:],
                                 start=first, stop=True)
                if final:
                    ot = opool.tile([P, DOUT_], f32, tag="o")
                    nc.scalar.activation(ot[:], ap[:],
                                         mybir.ActivationFunctionType.Copy)
                    nc.sync.dma_start(out_d[b * P:(b + 1) * P, :], ot[:])
                else:
                    hn = zpool.tile([P, P], f32, tag="hn")
                    nc.scalar.activation(hn[:], ap[:],
                                         mybir.ActivationFunctionType.Relu,
                                         scale=dinv_sb[:, b:b + 1])
                    tp = pt.tile([P, P], f32, tag="tp")
                    nc.tensor.transpose(out=tp[:], in_=hn[:], identity=ident[:])
                    nc.scalar.activation(hTnext[:, b * P:(b + 1) * P], tp[:],
                                         mybir.ActivationFunctionType.Copy)
                    if layer < 2:
                        # next layer's Z~ for this block; fire AGs when a
                        # half-slab completes
                        z_block(layer + 1, hTnext, b)
                        if b == H0B - 1:
                            ag(layer + 1, 0)
                        elif b == NB - 1:
                            ag(layer + 1, 1)
                    else:
                        # final head for this block
                        hp = pz.tile([P, DOUT_], f32, tag="z")
                        nc.tensor.matmul(hp[:],
                                         lhsT=hTnext[:, b * P:(b + 1) * P],
                                         rhs=wl_sb[:], start=True, stop=False)
                        nc.tensor.matmul(hp[:], lhsT=ones[:], rhs=bl_sb[:],
                                         start=False, stop=True)
                        ot = opool.tile([P, DOUT_], f32, tag="o")
                        nc.scalar.activation(ot[:], hp[:],
                                             mybir.ActivationFunctionType.Copy)
                        nc.sync.dma_start(out_d[b * P:(b + 1) * P, :], ot[:])

        # prologue: layer 0 Z~ from x
        for b in range(NB):
            z_block(0, hT[0], b)
            if b == H0B - 1:
                ag(0, 0)
            elif b == NB - 1:
                ag(0, 1)

        for layer in range(3):
            agg_phase(layer, hT[layer % 2], hT[(layer + 1) % 2])

    nc.compile()
    return nc


_CACHE = {}


def _get_compiled(edge_index):
    key = hash(np.asarray(edge_index, np.int64).tobytes())
    if key not in _CACHE:
        pp = _preprocess(edge_index, N_NODES, N_CORES)
        nc = _build(pp, DOUT, N_CORES)
        _CACHE[key] = (pp, nc)
    return _CACHE[key]


_LAST_RUN = {}


def kernel(x, edge_index, W1, b1, W2, b2, W3, b3, Wl, bl):
    x = np.asarray(x, np.float32)
    pp, nc = _get_compiled(edge_index)
    maps = _host_tensors(pp, x, (W1, b1, W2, b2, W3, b3, Wl, bl))

    from concourse.bass_utils import run_bass_kernel_spmd
    res = run_bass_kernel_spmd(nc, maps, core_ids=list(range(N_CORES)))
    LOCAL = pp["LOCAL"]
    parts = []
    for c in range(N_CORES):
        r = np.asarray(res.results[c]["out"])
        pm = pp["perm"][c]
        valid = pm >= 0
        o = np.zeros((LOCAL, r.shape[1]), r.dtype)
        o[pm[valid]] = r[valid]
        parts.append(o)
    out = np.concatenate(parts)
    _LAST_RUN["nc"] = nc
    _LAST_RUN["maps"] = maps
    return out


def _install_ntff_hook():
    """The agent image's antenv lacks axon_hooks; recreate it from the boot
    helper so run_bass_kernel_spmd(trace=True) can capture NTFF profiles."""
    import types
    if "antenv.axon_hooks" in sys.modules:
        return
    mod = types.ModuleType("antenv.axon_hooks")
    _state = {}
    mod.set_axon_ntff_profile_hook = lambda h: _state.__setitem__("h", h)
    mod.get_axon_ntff_profile_hook = lambda: _state.get("h")
    sys.modules["antenv.axon_hooks"] = mod
    import antenv
    antenv.axon_hooks = mod
    from trn_agent_boot.trn_boot import _ntff_profile_via_ctypes
    mod.set_axon_ntff_profile_hook(
        _ntff_profile_via_ctypes("/opt/axon/libaxon_pjrt.so"))


def profile_exec_ns():
    """Re-run the last kernel invocation with NTFF tracing; return exec ns."""
    if "nc" not in _LAST_RUN:
        return None
    _install_ntff_hook()
    from concourse.bass_utils import run_bass_kernel_spmd
    res = run_bass_kernel_spmd(
        _LAST_RUN["nc"], _LAST_RUN["maps"],
        core_ids=list(range(N_CORES)), trace=True,
    )
    _LAST_RUN["trace_res"] = res
    return res.exec_time_ns
